# revision 11
# baseline (speedup 1.0000x reference)
"""RWKV ChannelMixer (single-token) on 8 Trainium2 NeuronCores.

Math (reference):
    xn  = LayerNorm(x) * ln_w + ln_b
    xk  = xn*tmk + prev*(1-tmk);  xr = xn*tmr + prev*(1-tmr)
    r   = sigmoid(rw @ xr)                       # (D,)
    k   = relu(kw @ xk)^2                        # (F,)
    out = x + r * (vw @ k)                       # (D,)
    returns (out, xn)

Sharding (8 cores, no collectives -- cross-core sync costs ~60us here):
    kw: F-row-sharded (512 rows/core)  -> local k chunk (512,)
    vw: F-col-sharded (512 cols/core)  -> partial v_i = vw[:,Fi] @ k_i (1024,)
    rw: D-row-sharded (128 rows/core)  -> r chunk (128,)
    LN/mix replicated.  Host unshard: v = sum_i v_i, r = concat(r_i),
    out = x + r*v.

Engines: dot-products run on the Vector engine (tensor_tensor_reduce,
fp32 @ ~1 elem/lane/cycle ~ 490GB/s > 358GB/s HBM/core, so the kernel
stays DMA-bound).  TensorE only does tiny selector-matmul broadcasts /
transposes.  Weight matrices stream through SBUF in natural row-major
layout (host reshapes rows onto 128 partitions; no transposes).
"""

import sys
import numpy as np

for _p in ("/opt/trn_rl_repo", "/root/.axon_site/_ro/trn_rl_repo"):
    if _p not in sys.path:
        sys.path.append(_p)

D = 1024
F = 4096
N_CORES = 8
FSH = F // N_CORES      # 512 kw rows / vw cols per core
DSH = D // N_CORES      # 128 rw rows per core
LN_EPS = 1e-5

_STATE = {}


def _body(nc, tc, mybir, stage):
    f32 = mybir.dt.float32
    Alu = mybir.AluOpType
    Act = mybir.ActivationFunctionType
    AxX = mybir.AxisListType.X

    kw_d = nc.dram_tensor("kw_p", [128, 4096], f32, kind="ExternalInput").ap()
    vw_d = nc.dram_tensor("vw_p", [128, 4096], f32, kind="ExternalInput").ap()
    rw_d = nc.dram_tensor("rw_p", [128, 1024], f32, kind="ExternalInput").ap()
    # x, prev, tmk, tmr, lnw, lnb stacked: [8, 6*128], row j = vectors' d-slice j
    sm_d = nc.dram_tensor("smalls", [8, 768], f32, kind="ExternalInput").ap()

    xn_d = nc.dram_tensor("xn_out", [8, 128], f32, kind="ExternalOutput").ap()
    v_d = nc.dram_tensor("v_out", [8, 128], f32, kind="ExternalOutput").ap()
    r_d = nc.dram_tensor("r_out", [1, 128], f32, kind="ExternalOutput").ap()

    import contextlib
    with contextlib.ExitStack() as ctx:
        wp = ctx.enter_context(tc.tile_pool(name="w", bufs=1))
        vp = ctx.enter_context(tc.tile_pool(name="v", bufs=1))
        bp = ctx.enter_context(tc.tile_pool(name="bc", bufs=2, space="PSUM"))
        pp = ctx.enter_context(tc.tile_pool(name="ps", bufs=1, space="PSUM"))
        dp = ctx.enter_context(tc.tile_pool(name="dr", bufs=1, space="DRAM"))

        # ---- small packed DMA first, then bulk (same HWDGE FIFO: sm->kw->rw->vw)
        sm_sb = vp.tile([8, 768], f32, tag="sm")
        nc.sync.dma_start(out=sm_sb[:], in_=sm_d[:])
        x_row = sm_sb[:, 0:128]
        pv_row = sm_sb[:, 128:256]
        tk_row = sm_sb[:, 256:384]
        tr_row = sm_sb[:, 384:512]
        lw_row = sm_sb[:, 512:640]
        lb_row = sm_sb[:, 640:768]

        if stage >= 2:
            kw_sb = wp.tile([128, 4096], f32, tag="kw")
            rw_sb = wp.tile([128, 1024], f32, tag="rw")
            vw_sb = wp.tile([128, 4096], f32, tag="vw")
            for c in range(4):
                nc.sync.dma_start(out=kw_sb[:, c * 1024:(c + 1) * 1024],
                                  in_=kw_d[:, c * 1024:(c + 1) * 1024])
            nc.sync.dma_start(out=rw_sb[:], in_=rw_d[:])
            for c in range(4):
                nc.sync.dma_start(out=vw_sb[:, c * 1024:(c + 1) * 1024],
                                  in_=vw_d[:, c * 1024:(c + 1) * 1024])

        # ---- constants
        ones_c8 = vp.tile([8, 1], f32, tag="ones_c8")
        ones_r8 = vp.tile([1, 8], f32, tag="ones_r8")
        eps_t = vp.tile([1, 1], f32, tag="eps")
        dummy_t = vp.tile([1, 1], f32, tag="dummy")
        nc.vector.memset(ones_c8[:], 1.0)
        nc.vector.memset(ones_r8[:], 1.0)
        nc.vector.memset(eps_t[:], LN_EPS)
        # pre-warm the ACT Sqrt table while DMAs stream (off critical path)
        nc.scalar.activation(dummy_t[:], eps_t[:], Act.Sqrt)
        if stage >= 3:
            # one-hot row-selector matrices (lhsT for row-broadcast matmuls)
            sel8 = vp.tile([8, 1024], f32, tag="sel8")
            sel4 = vp.tile([4, 512], f32, tag="sel4")
            nc.gpsimd.memset(sel8[:], 0.0)
            nc.gpsimd.memset(sel4[:], 0.0)
            nc.gpsimd.affine_select(
                out=sel8[:].rearrange("p (j q) -> p j q", j=8),
                in_=sel8[:].rearrange("p (j q) -> p j q", j=8),
                compare_op=Alu.not_equal, fill=1.0, base=0,
                pattern=[[-1, 8], [0, 128]], channel_multiplier=1)
            nc.gpsimd.affine_select(
                out=sel4[:].rearrange("p (j q) -> p j q", j=4),
                in_=sel4[:].rearrange("p (j q) -> p j q", j=4),
                compare_op=Alu.not_equal, fill=1.0, base=0,
                pattern=[[-1, 4], [0, 128]], channel_multiplier=1)

        # ---- LayerNorm stats over 1024 elems laid out [8, 128]
        s2 = vp.tile([8, 2], f32, tag="s2")
        xsq = vp.tile([8, 128], f32, tag="xsq")
        nc.vector.tensor_reduce(out=s2[:, 0:1], in_=x_row, axis=AxX, op=Alu.add)
        nc.vector.scalar_tensor_tensor(out=xsq[:], in0=x_row, scalar=1.0,
                                       in1=x_row, op0=Alu.mult, op1=Alu.mult,
                                       accum_out=s2[:, 1:2])

        # B0 = prev*(1-tm), computed while the PE stats roundtrip is in flight
        b0k = vp.tile([8, 128], f32, tag="b0k")
        b0r = vp.tile([8, 128], f32, tag="b0r")

        psum_s = pp.tile([1, 2], f32, tag="pmisc", bufs=2)
        nc.tensor.matmul(psum_s[:], ones_c8[:], s2[:], start=True, stop=True)
        nc.vector.tensor_mul(b0k[:], pv_row, tk_row)
        nc.vector.tensor_sub(b0k[:], pv_row, b0k[:])
        nc.vector.tensor_mul(b0r[:], pv_row, tr_row)
        nc.vector.tensor_sub(b0r[:], pv_row, b0r[:])
        ssum = vp.tile([1, 2], f32, tag="ssum")     # [mean, E[x^2]]
        nc.scalar.mul(ssum[:], psum_s[:], 1.0 / D)

        mr = vp.tile([1, 2], f32, tag="mr")         # [mean, rstd]
        var_t = vp.tile([1, 1], f32, tag="var")
        std_t = vp.tile([1, 1], f32, tag="std")
        nc.vector.tensor_tensor(mr[:, 0:1], ssum[:, 0:1], ssum[:, 0:1], Alu.mult)
        nc.vector.tensor_tensor(var_t[:], ssum[:, 1:2], mr[:, 0:1], Alu.subtract)
        nc.scalar.activation(std_t[:], var_t[:], Act.Sqrt, bias=eps_t[:])
        nc.vector.reciprocal(mr[:, 1:2], std_t[:])
        nc.scalar.copy(mr[:, 0:1], ssum[:, 0:1])

        psum_b = pp.tile([8, 2], f32, tag="pmisc", bufs=2)
        nc.tensor.matmul(psum_b[:], ones_r8[:], mr[:], start=True, stop=True)
        bc8 = vp.tile([8, 2], f32, tag="bc8")
        nc.scalar.copy(bc8[:], psum_b[:])

        xn_row = vp.tile([8, 128], f32, tag="xn")
        nc.vector.tensor_scalar(out=xn_row[:], in0=x_row,
                                scalar1=bc8[:, 0:1], scalar2=bc8[:, 1:2],
                                op0=Alu.subtract, op1=Alu.mult)
        nc.vector.tensor_mul(xn_row[:], xn_row[:], lw_row)
        nc.vector.tensor_add(xn_row[:], xn_row[:], lb_row)
        nc.gpsimd.dma_start(out=xn_d[:], in_=xn_row[:])

        # ---- token mixes: xk = tmk*xn + prev*(1-tmk)  (B0 precomputed above)
        xk_row = vp.tile([8, 128], f32, tag="xk")
        xr_row = vp.tile([8, 128], f32, tag="xr")
        nc.vector.tensor_mul(xk_row[:], xn_row[:], tk_row)
        nc.vector.tensor_add(xk_row[:], xk_row[:], b0k[:])
        nc.vector.tensor_mul(xr_row[:], xn_row[:], tr_row)
        nc.vector.tensor_add(xr_row[:], xr_row[:], b0r[:])

        if stage < 3:
            return

        # ---- broadcast xk across partitions: [8,128] -> [128, 1024]
        xk_bc = vp.tile([128, 1024], f32, tag="xk_bc")
        for j in range(8):
            pb = bp.tile([128, 128], f32, tag="pb", name=f"pbk{j}")
            nc.tensor.matmul(pb[:], sel8[:, j * 128:(j + 1) * 128], xk_row[:],
                             start=True, stop=True)
            nc.scalar.copy(xk_bc[:, j * 128:(j + 1) * 128], pb[:])

        # pre-warm the Sigmoid table during the kw-dot window
        nc.scalar.activation(dummy_t[:], eps_t[:], Act.Sigmoid)

        if stage < 4:
            return

        # ---- stage A: k chunk = sqrelu(kw_i @ xk); kw tile c = rows 128c..
        scratch = vp.tile([128, 1024], f32, tag="scratch")
        k_sb = vp.tile([128, 4], f32, tag="k")
        for c in range(4):
            nc.vector.scalar_tensor_tensor(
                out=scratch[:], in0=kw_sb[:, c * 1024:(c + 1) * 1024],
                scalar=1.0, in1=xk_bc[:],
                op0=Alu.mult, op1=Alu.mult, accum_out=k_sb[:, c:c + 1])
        krelu = vp.tile([128, 4], f32, tag="krelu")
        ksq = vp.tile([128, 4], f32, tag="ksq")
        nc.vector.tensor_scalar_max(krelu[:], k_sb[:], 0.0)
        nc.vector.tensor_mul(ksq[:], krelu[:], krelu[:])

        if stage < 5:
            return

        # ---- broadcast xr (during kw dots) and compute r
        xr_bc = vp.tile([128, 1024], f32, tag="xr_bc")
        for j in range(8):
            pb = bp.tile([128, 128], f32, tag="pb", name=f"pbr{j}")
            nc.tensor.matmul(pb[:], sel8[:, j * 128:(j + 1) * 128], xr_row[:],
                             start=True, stop=True)
            nc.scalar.copy(xr_bc[:, j * 128:(j + 1) * 128], pb[:])

        pre_r = vp.tile([128, 1], f32, tag="pre_r")
        nc.vector.scalar_tensor_tensor(
            out=scratch[:], in0=rw_sb[:], scalar=1.0, in1=xr_bc[:],
            op0=Alu.mult, op1=Alu.mult, accum_out=pre_r[:])
        r_sb = vp.tile([128, 1], f32, tag="r")
        nc.scalar.activation(r_sb[:], pre_r[:], Act.Sigmoid)

        if stage < 6:
            return

        # ---- k broadcast: PE transpose -> DRAM -> replicated load [128, 512]
        from concourse.masks import make_identity
        ident = vp.tile([128, 128], f32, tag="ident")
        make_identity(nc, ident)
        kT_ps = pp.tile([4, 128], f32, tag="pmisc", bufs=2)
        nc.tensor.transpose(kT_ps[:], ksq[:], ident[:])
        kT = vp.tile([4, 128], f32, tag="kT")
        nc.scalar.copy(kT[:], kT_ps[:])
        kd = dp.tile([4, 128], f32, tag="kd")
        nc.sync.dma_start(out=kd[:], in_=kT[:])
        k_bc = vp.tile([128, 512], f32, tag="k_bc")
        nc.sync.dma_start(
            out=k_bc[:],
            in_=kd[:].rearrange("c q -> (c q)").partition_broadcast(128))

        # ---- stage V: v partial, 8 d-chunks of [128, 512] x k_bc
        v_sb = vp.tile([128, 8], f32, tag="v")
        for m in range(8):
            nc.vector.scalar_tensor_tensor(
                out=scratch[:, 0:512], in0=vw_sb[:, m * 512:(m + 1) * 512],
                scalar=1.0, in1=k_bc[:],
                op0=Alu.mult, op1=Alu.mult, accum_out=v_sb[:, m:m + 1])

        # ---- outputs in row form (contiguous DMA): transpose via PE
        vT_ps = pp.tile([8, 128], f32, tag="pmisc", bufs=2)
        nc.tensor.transpose(vT_ps[:], v_sb[:], ident[:])
        vT = vp.tile([8, 128], f32, tag="vT")
        nc.scalar.copy(vT[:], vT_ps[:])
        nc.gpsimd.dma_start(out=v_d[:], in_=vT[:])

        rT_ps = pp.tile([1, 128], f32, tag="pmisc", bufs=2)
        nc.tensor.transpose(rT_ps[:], r_sb[:], ident[:])
        rT = vp.tile([1, 128], f32, tag="rT")
        nc.scalar.copy(rT[:], rT_ps[:])
        nc.gpsimd.dma_start(out=r_d[:], in_=rT[:])


def _build(stage=6):
    import concourse.bacc as bacc
    import concourse.tile as tile
    from concourse import mybir

    nc = bacc.Bacc("TRN2", target_bir_lowering=False, debug=False,
                   num_devices=N_CORES)
    with tile.TileContext(nc) as tc:
        _body(nc, tc, mybir, stage)
    nc.compile()
    return nc


def _prep_shared(kw, vw, rw):
    """Slice + reshape weights per core (rows onto 128 partitions)."""
    kw_p, vw_p, rw_p = [], [], []
    for i in range(N_CORES):
        A = kw[i * FSH:(i + 1) * FSH, :]                # (512, 1024) rows f
        A = A.reshape(4, 128, 1024).transpose(1, 0, 2)  # [p, c, d]
        kw_p.append(np.ascontiguousarray(A.reshape(128, 4096)))

        B = rw[i * DSH:(i + 1) * DSH, :]                # (128, 1024) rows d
        rw_p.append(np.ascontiguousarray(B))

        C = vw[:, i * FSH:(i + 1) * FSH]                # (1024, 512) rows d
        C = C.reshape(8, 128, FSH).transpose(1, 0, 2)   # [p, m, f]
        vw_p.append(np.ascontiguousarray(C.reshape(128, 4096)))
    return kw_p, vw_p, rw_p


def _prep_smalls(x, state, tmk, tmr, lnw, lnb):
    sm = np.stack([x.reshape(8, 128), state[0].reshape(8, 128),
                   tmk.reshape(8, 128), tmr.reshape(8, 128),
                   lnw.reshape(8, 128), lnb.reshape(8, 128)], axis=1)
    return np.ascontiguousarray(sm.reshape(8, 768))


def kernel(x, state, time_mix_k, time_mix_r, kw, vw, rw, ln_weight, ln_bias):
    from concourse import bass_utils

    x = np.asarray(x, dtype=np.float32)
    state = np.asarray(state, dtype=np.float32)
    kw = np.asarray(kw, dtype=np.float32)
    vw = np.asarray(vw, dtype=np.float32)
    rw = np.asarray(rw, dtype=np.float32)
    tmk = np.asarray(time_mix_k, dtype=np.float32)
    tmr = np.asarray(time_mix_r, dtype=np.float32)
    lnw = np.asarray(ln_weight, dtype=np.float32)
    lnb = np.asarray(ln_bias, dtype=np.float32)

    if "nc" not in _STATE:
        _STATE["nc"] = _build()
    nc = _STATE["nc"]

    kw_p, vw_p, rw_p = _prep_shared(kw, vw, rw)
    sm = _prep_smalls(x, state, tmk, tmr, lnw, lnb)

    in_maps = [{"kw_p": kw_p[i], "vw_p": vw_p[i], "rw_p": rw_p[i], "smalls": sm}
               for i in range(N_CORES)]

    res = bass_utils.run_bass_kernel_spmd(nc, in_maps, core_ids=list(range(N_CORES)))

    # unshard: v = sum of partials, r = concat of chunks
    v = np.zeros(D, dtype=np.float64)
    for i in range(N_CORES):
        v += res.results[i]["v_out"].reshape(D).astype(np.float64)
    r = np.concatenate([res.results[i]["r_out"].reshape(DSH)
                        for i in range(N_CORES)])
    out = x + r * v.astype(np.float32)
    xn = res.results[0]["xn_out"].reshape(D)
    return np.asarray(out, dtype=np.float32), np.asarray(xn, dtype=np.float32)


# revision 12
# speedup vs baseline: 1.0431x; 1.0431x over previous
"""RWKV ChannelMixer (single-token) on 8 Trainium2 NeuronCores.

Math (reference):
    xn  = LayerNorm(x) * ln_w + ln_b
    xk  = xn*tmk + prev*(1-tmk);  xr = xn*tmr + prev*(1-tmr)
    r   = sigmoid(rw @ xr)                       # (D,)
    k   = relu(kw @ xk)^2                        # (F,)
    out = x + r * (vw @ k)                       # (D,)
    returns (out, xn)

Sharding (8 cores, no collectives -- cross-core sync costs ~60us here):
    kw: F-row-sharded (512 rows/core)  -> local k chunk (512,)
    vw: F-col-sharded (512 cols/core)  -> partial v_i = vw[:,Fi] @ k_i (1024,)
    rw: D-row-sharded (128 rows/core)  -> r chunk (128,)
    LN/mix replicated.  Host unshard: v = sum_i v_i, r = concat(r_i),
    out = x + r*v.

Engines: dot-products run on the Vector engine (tensor_tensor_reduce,
fp32 @ ~1 elem/lane/cycle ~ 490GB/s > 358GB/s HBM/core, so the kernel
stays DMA-bound).  TensorE only does tiny selector-matmul broadcasts /
transposes.  Weight matrices stream through SBUF in natural row-major
layout (host reshapes rows onto 128 partitions; no transposes).
"""

import sys
import numpy as np

for _p in ("/opt/trn_rl_repo", "/root/.axon_site/_ro/trn_rl_repo"):
    if _p not in sys.path:
        sys.path.append(_p)

D = 1024
F = 4096
N_CORES = 8
FSH = F // N_CORES      # 512 kw rows / vw cols per core
DSH = D // N_CORES      # 128 rw rows per core
LN_EPS = 1e-5

_STATE = {}


def _body(nc, tc, mybir, stage):
    f32 = mybir.dt.float32
    Alu = mybir.AluOpType
    Act = mybir.ActivationFunctionType
    AxX = mybir.AxisListType.X

    kw_d = nc.dram_tensor("kw_p", [128, 4096], f32, kind="ExternalInput").ap()
    vw_d = nc.dram_tensor("vw_p", [128, 4096], f32, kind="ExternalInput").ap()
    rw_d = nc.dram_tensor("rw_p", [128, 1024], f32, kind="ExternalInput").ap()
    # x, prev, tmk, tmr, lnw, lnb stacked: [8, 6*128], row j = vectors' d-slice j
    sm_d = nc.dram_tensor("smalls", [8, 768], f32, kind="ExternalInput").ap()

    xn_d = nc.dram_tensor("xn_out", [8, 128], f32, kind="ExternalOutput").ap()
    v_d = nc.dram_tensor("v_out", [8, 128], f32, kind="ExternalOutput").ap()
    r_d = nc.dram_tensor("r_out", [1, 128], f32, kind="ExternalOutput").ap()

    import contextlib
    with contextlib.ExitStack() as ctx:
        wp = ctx.enter_context(tc.tile_pool(name="w", bufs=1))
        vp = ctx.enter_context(tc.tile_pool(name="v", bufs=1))
        bp = ctx.enter_context(tc.tile_pool(name="bc", bufs=2, space="PSUM"))
        pp = ctx.enter_context(tc.tile_pool(name="ps", bufs=1, space="PSUM"))
        dp = ctx.enter_context(tc.tile_pool(name="dr", bufs=1, space="DRAM"))

        # ---- small packed DMA first, then bulk (same HWDGE FIFO: sm->kw->rw->vw)
        sm_sb = vp.tile([8, 768], f32, tag="sm")
        nc.sync.dma_start(out=sm_sb[:], in_=sm_d[:])
        x_row = sm_sb[:, 0:128]
        pv_row = sm_sb[:, 128:256]
        tk_row = sm_sb[:, 256:384]
        tr_row = sm_sb[:, 384:512]
        lw_row = sm_sb[:, 512:640]
        lb_row = sm_sb[:, 640:768]

        if stage >= 2:
            kw_sb = wp.tile([128, 4096], f32, tag="kw")
            rw_sb = wp.tile([128, 1024], f32, tag="rw")
            vw_sb = wp.tile([128, 4096], f32, tag="vw")
            for c in range(4):
                nc.sync.dma_start(out=kw_sb[:, c * 1024:(c + 1) * 1024],
                                  in_=kw_d[:, c * 1024:(c + 1) * 1024])
            nc.sync.dma_start(out=rw_sb[:], in_=rw_d[:])
            for c in range(4):
                nc.sync.dma_start(out=vw_sb[:, c * 1024:(c + 1) * 1024],
                                  in_=vw_d[:, c * 1024:(c + 1) * 1024])

        # ---- constants
        ones_c8 = vp.tile([8, 1], f32, tag="ones_c8")
        ones_r8 = vp.tile([1, 8], f32, tag="ones_r8")
        eps_t = vp.tile([1, 1], f32, tag="eps")
        nc.vector.memset(ones_c8[:], 1.0)
        nc.vector.memset(ones_r8[:], 1.0)
        nc.vector.memset(eps_t[:], LN_EPS)
        warm_lhs = vp.tile([8, 128], f32, tag="warm_lhs")
        warm_rhs = vp.tile([8, 256], f32, tag="warm_rhs")
        nc.vector.memset(warm_lhs[:], 1.0)
        nc.vector.memset(warm_rhs[:], 1.0)

        if stage >= 3:
            # one-hot row-selector matrices (lhsT for row-broadcast matmuls)
            sel8 = vp.tile([8, 1024], f32, tag="sel8")
            sel4 = vp.tile([4, 512], f32, tag="sel4")
            nc.gpsimd.memset(sel8[:], 0.0)
            nc.gpsimd.memset(sel4[:], 0.0)
            nc.gpsimd.affine_select(
                out=sel8[:].rearrange("p (j q) -> p j q", j=8),
                in_=sel8[:].rearrange("p (j q) -> p j q", j=8),
                compare_op=Alu.not_equal, fill=1.0, base=0,
                pattern=[[-1, 8], [0, 128]], channel_multiplier=1)
            nc.gpsimd.affine_select(
                out=sel4[:].rearrange("p (j q) -> p j q", j=4),
                in_=sel4[:].rearrange("p (j q) -> p j q", j=4),
                compare_op=Alu.not_equal, fill=1.0, base=0,
                pattern=[[-1, 4], [0, 128]], channel_multiplier=1)

        # ---- LayerNorm stats over 1024 elems laid out [8, 128]
        s2 = vp.tile([8, 2], f32, tag="s2")
        xsq = vp.tile([8, 128], f32, tag="xsq")
        nc.vector.tensor_reduce(out=s2[:, 0:1], in_=x_row, axis=AxX, op=Alu.add)
        nc.vector.scalar_tensor_tensor(out=xsq[:], in0=x_row, scalar=1.0,
                                       in1=x_row, op0=Alu.mult, op1=Alu.mult,
                                       accum_out=s2[:, 1:2])

        # Mix coefficients, computed while the PE stats roundtrip is in
        # flight:  xk = tmk*xn + prev*(1-tmk)  with  xn = xn_pre*lnw + lnb
        #       => xk = Ck*xn_pre + Ek,  Ck = tmk*lnw,
        #          Ek = tmk*lnb + prev*(1-tmk)
        b0k = vp.tile([8, 128], f32, tag="b0k")
        b0r = vp.tile([8, 128], f32, tag="b0r")
        ck = vp.tile([8, 128], f32, tag="ck")
        cr = vp.tile([8, 128], f32, tag="cr")
        ek = vp.tile([8, 128], f32, tag="ek")
        er = vp.tile([8, 128], f32, tag="er")

        # keep the PE busy so HAM unthrottles (1.2 -> 2.4 GHz) before the
        # broadcast matmuls; junk pairs run during the DMA/LN wait window
        for w in range(2):
            wp_ps = bp.tile([128, 256], f32, tag="warm", name=f"warm{w}", bufs=1)
            nc.tensor.matmul(wp_ps[:], warm_lhs[:], warm_rhs[:], start=True, stop=True)

        psum_s = pp.tile([1, 2], f32, tag="pmisc", bufs=2)
        nc.tensor.matmul(psum_s[:], ones_c8[:], s2[:], start=True, stop=True)
        for w in range(2, 5):
            wp_ps = bp.tile([128, 256], f32, tag="warm", name=f"warm{w}", bufs=1)
            nc.tensor.matmul(wp_ps[:], warm_lhs[:], warm_rhs[:], start=True, stop=True)
        nc.vector.tensor_mul(b0k[:], pv_row, tk_row)
        nc.vector.tensor_sub(b0k[:], pv_row, b0k[:])
        nc.vector.tensor_mul(b0r[:], pv_row, tr_row)
        nc.vector.tensor_sub(b0r[:], pv_row, b0r[:])
        nc.vector.tensor_mul(ck[:], tk_row, lw_row)
        nc.vector.tensor_mul(cr[:], tr_row, lw_row)
        nc.vector.tensor_mul(ek[:], tk_row, lb_row)
        nc.vector.tensor_add(ek[:], ek[:], b0k[:])
        nc.vector.tensor_mul(er[:], tr_row, lb_row)
        nc.vector.tensor_add(er[:], er[:], b0r[:])
        ssum = vp.tile([1, 2], f32, tag="ssum")     # [mean, E[x^2]]
        nc.scalar.mul(ssum[:], psum_s[:], 1.0 / D)

        mr = vp.tile([1, 2], f32, tag="mr")         # [mean, rstd]
        var_t = vp.tile([1, 1], f32, tag="var")
        std_t = vp.tile([1, 1], f32, tag="std")
        nc.vector.tensor_tensor(mr[:, 0:1], ssum[:, 0:1], ssum[:, 0:1], Alu.mult)
        nc.vector.tensor_tensor(var_t[:], ssum[:, 1:2], mr[:, 0:1], Alu.subtract)
        nc.scalar.activation(std_t[:], var_t[:], Act.Sqrt, bias=eps_t[:])
        nc.vector.reciprocal(mr[:, 1:2], std_t[:])
        nc.scalar.copy(mr[:, 0:1], ssum[:, 0:1])

        psum_b = pp.tile([8, 2], f32, tag="pmisc", bufs=2)
        nc.tensor.matmul(psum_b[:], ones_r8[:], mr[:], start=True, stop=True)
        for w in range(5, 7):
            wp_ps = bp.tile([128, 256], f32, tag="warm", name=f"warm{w}", bufs=1)
            nc.tensor.matmul(wp_ps[:], warm_lhs[:], warm_rhs[:], start=True, stop=True)
        bc8 = vp.tile([8, 2], f32, tag="bc8")
        nc.scalar.copy(bc8[:], psum_b[:])

        xn_pre = vp.tile([8, 128], f32, tag="xn_pre")
        nc.vector.tensor_scalar(out=xn_pre[:], in0=x_row,
                                scalar1=bc8[:, 0:1], scalar2=bc8[:, 1:2],
                                op0=Alu.subtract, op1=Alu.mult)

        # ---- token mixes straight from xn_pre (critical path)
        xk_row = vp.tile([8, 128], f32, tag="xk")
        xr_row = vp.tile([8, 128], f32, tag="xr")
        nc.vector.tensor_mul(xk_row[:], xn_pre[:], ck[:])
        nc.vector.tensor_add(xk_row[:], xk_row[:], ek[:])
        nc.vector.tensor_mul(xr_row[:], xn_pre[:], cr[:])
        nc.vector.tensor_add(xr_row[:], xr_row[:], er[:])

        # full xn only feeds the output (off the critical path)
        xn_row = vp.tile([8, 128], f32, tag="xn")
        nc.vector.tensor_mul(xn_row[:], xn_pre[:], lw_row)
        nc.vector.tensor_add(xn_row[:], xn_row[:], lb_row)
        nc.sync.dma_start(out=xn_d[:], in_=xn_row[:])

        if stage < 3:
            return

        # ---- broadcast xk across partitions: [8,128] -> [128, 1024]
        xk_bc = vp.tile([128, 1024], f32, tag="xk_bc")
        for j in range(8):
            pb = bp.tile([128, 128], f32, tag="pb", name=f"pbk{j}")
            nc.tensor.matmul(pb[:], sel8[:, j * 128:(j + 1) * 128], xk_row[:],
                             start=True, stop=True)
            nc.scalar.copy(xk_bc[:, j * 128:(j + 1) * 128], pb[:])

        if stage < 4:
            return

        # ---- stage A: k chunk = sqrelu(kw_i @ xk); kw tile c = rows 128c..
        scratch = vp.tile([128, 1024], f32, tag="scratch")
        k_sb = vp.tile([128, 4], f32, tag="k")
        for c in range(4):
            nc.vector.scalar_tensor_tensor(
                out=scratch[:], in0=kw_sb[:, c * 1024:(c + 1) * 1024],
                scalar=1.0, in1=xk_bc[:],
                op0=Alu.mult, op1=Alu.mult, accum_out=k_sb[:, c:c + 1])
        krelu = vp.tile([128, 4], f32, tag="krelu")
        ksq = vp.tile([128, 4], f32, tag="ksq")
        nc.vector.tensor_scalar_max(krelu[:], k_sb[:], 0.0)
        nc.vector.tensor_mul(ksq[:], krelu[:], krelu[:])

        if stage < 5:
            return

        # ---- broadcast xr (during kw dots) and compute r
        xr_bc = vp.tile([128, 1024], f32, tag="xr_bc")
        for j in range(8):
            pb = bp.tile([128, 128], f32, tag="pb", name=f"pbr{j}")
            nc.tensor.matmul(pb[:], sel8[:, j * 128:(j + 1) * 128], xr_row[:],
                             start=True, stop=True)
            nc.scalar.copy(xr_bc[:, j * 128:(j + 1) * 128], pb[:])

        pre_r = vp.tile([128, 1], f32, tag="pre_r")
        nc.vector.scalar_tensor_tensor(
            out=scratch[:], in0=rw_sb[:], scalar=1.0, in1=xr_bc[:],
            op0=Alu.mult, op1=Alu.mult, accum_out=pre_r[:])
        r_sb = vp.tile([128, 1], f32, tag="r")
        nc.scalar.activation(r_sb[:], pre_r[:], Act.Sigmoid)

        if stage < 6:
            return

        # ---- k broadcast: PE transpose, then 4 selector matmuls into one
        #      PSUM bank (vw dots read it straight from PSUM)
        from concourse.masks import make_identity
        ident = vp.tile([128, 128], f32, tag="ident")
        make_identity(nc, ident)
        kT_ps = pp.tile([4, 128], f32, tag="pmisc", bufs=2)
        nc.tensor.transpose(kT_ps[:], ksq[:], ident[:])
        kT = vp.tile([4, 128], f32, tag="kT")
        nc.scalar.copy(kT[:], kT_ps[:])
        k_bc = pp.tile([128, 512], f32, tag="kbc_ps", bufs=1)
        for c in range(4):
            nc.tensor.matmul(k_bc[:, c * 128:(c + 1) * 128],
                             sel4[:, c * 128:(c + 1) * 128], kT[:],
                             start=True, stop=True)

        # ---- stage V: v partial, 8 d-chunks of [128, 512] x k_bc
        v_sb = vp.tile([128, 8], f32, tag="v")
        for m in range(8):
            nc.vector.scalar_tensor_tensor(
                out=scratch[:, 0:512], in0=vw_sb[:, m * 512:(m + 1) * 512],
                scalar=1.0, in1=k_bc[:],
                op0=Alu.mult, op1=Alu.mult, accum_out=v_sb[:, m:m + 1])

        # ---- outputs in row form (contiguous DMA): transpose via PE
        vT_ps = pp.tile([8, 128], f32, tag="pmisc", bufs=2)
        nc.tensor.transpose(vT_ps[:], v_sb[:], ident[:])
        vT = vp.tile([8, 128], f32, tag="vT")
        nc.scalar.copy(vT[:], vT_ps[:])
        nc.sync.dma_start(out=v_d[:], in_=vT[:])

        rT_ps = pp.tile([1, 128], f32, tag="pmisc", bufs=2)
        nc.tensor.transpose(rT_ps[:], r_sb[:], ident[:])
        rT = vp.tile([1, 128], f32, tag="rT")
        nc.scalar.copy(rT[:], rT_ps[:])
        nc.sync.dma_start(out=r_d[:], in_=rT[:])


def _build(stage=6):
    import concourse.bacc as bacc
    import concourse.tile as tile
    from concourse import mybir

    nc = bacc.Bacc("TRN2", target_bir_lowering=False, debug=False,
                   num_devices=N_CORES)
    with tile.TileContext(nc) as tc:
        _body(nc, tc, mybir, stage)
    nc.compile()
    return nc


def _prep_shared(kw, vw, rw):
    """Slice + reshape weights per core (rows onto 128 partitions)."""
    kw_p, vw_p, rw_p = [], [], []
    for i in range(N_CORES):
        A = kw[i * FSH:(i + 1) * FSH, :]                # (512, 1024) rows f
        A = A.reshape(4, 128, 1024).transpose(1, 0, 2)  # [p, c, d]
        kw_p.append(np.ascontiguousarray(A.reshape(128, 4096)))

        B = rw[i * DSH:(i + 1) * DSH, :]                # (128, 1024) rows d
        rw_p.append(np.ascontiguousarray(B))

        C = vw[:, i * FSH:(i + 1) * FSH]                # (1024, 512) rows d
        C = C.reshape(8, 128, FSH).transpose(1, 0, 2)   # [p, m, f]
        vw_p.append(np.ascontiguousarray(C.reshape(128, 4096)))
    return kw_p, vw_p, rw_p


def _prep_smalls(x, state, tmk, tmr, lnw, lnb):
    sm = np.stack([x.reshape(8, 128), state[0].reshape(8, 128),
                   tmk.reshape(8, 128), tmr.reshape(8, 128),
                   lnw.reshape(8, 128), lnb.reshape(8, 128)], axis=1)
    return np.ascontiguousarray(sm.reshape(8, 768))


def kernel(x, state, time_mix_k, time_mix_r, kw, vw, rw, ln_weight, ln_bias):
    from concourse import bass_utils

    x = np.asarray(x, dtype=np.float32)
    state = np.asarray(state, dtype=np.float32)
    kw = np.asarray(kw, dtype=np.float32)
    vw = np.asarray(vw, dtype=np.float32)
    rw = np.asarray(rw, dtype=np.float32)
    tmk = np.asarray(time_mix_k, dtype=np.float32)
    tmr = np.asarray(time_mix_r, dtype=np.float32)
    lnw = np.asarray(ln_weight, dtype=np.float32)
    lnb = np.asarray(ln_bias, dtype=np.float32)

    if "nc" not in _STATE:
        _STATE["nc"] = _build()
    nc = _STATE["nc"]

    kw_p, vw_p, rw_p = _prep_shared(kw, vw, rw)
    sm = _prep_smalls(x, state, tmk, tmr, lnw, lnb)

    in_maps = [{"kw_p": kw_p[i], "vw_p": vw_p[i], "rw_p": rw_p[i], "smalls": sm}
               for i in range(N_CORES)]

    res = bass_utils.run_bass_kernel_spmd(nc, in_maps, core_ids=list(range(N_CORES)))

    # unshard: v = sum of partials, r = concat of chunks
    v = np.zeros(D, dtype=np.float64)
    for i in range(N_CORES):
        v += res.results[i]["v_out"].reshape(D).astype(np.float64)
    r = np.concatenate([res.results[i]["r_out"].reshape(DSH)
                        for i in range(N_CORES)])
    out = x + r * v.astype(np.float32)
    xn = res.results[0]["xn_out"].reshape(D)
    return np.asarray(out, dtype=np.float32), np.asarray(xn, dtype=np.float32)


# revision 13
# speedup vs baseline: 1.1266x; 1.0800x over previous
"""RWKV ChannelMixer (single-token) on 8 Trainium2 NeuronCores.

Math (reference):
    xn  = LayerNorm(x) * ln_w + ln_b
    xk  = xn*tmk + prev*(1-tmk);  xr = xn*tmr + prev*(1-tmr)
    r   = sigmoid(rw @ xr)                       # (D,)
    k   = relu(kw @ xk)^2                        # (F,)
    out = x + r * (vw @ k)                       # (D,)
    returns (out, xn)

Sharding (8 cores, no collectives -- cross-core sync costs ~60us here):
    kw: F-row-sharded (512 rows/core)  -> local k chunk (512,)
    vw: F-col-sharded (512 cols/core)  -> partial v_i = vw[:,Fi] @ k_i (1024,)
    rw: D-row-sharded (128 rows/core)  -> r chunk (128,)
    LN/mix replicated.  Host unshard: v = sum_i v_i, r = concat(r_i),
    out = x + r*v.

Engines: dot-products run on the Vector engine (tensor_tensor_reduce,
fp32 @ ~1 elem/lane/cycle ~ 490GB/s > 358GB/s HBM/core, so the kernel
stays DMA-bound).  TensorE only does tiny selector-matmul broadcasts /
transposes.  Weight matrices stream through SBUF in natural row-major
layout (host reshapes rows onto 128 partitions; no transposes).
"""

import sys
import numpy as np

for _p in ("/opt/trn_rl_repo", "/root/.axon_site/_ro/trn_rl_repo"):
    if _p not in sys.path:
        sys.path.append(_p)

D = 1024
F = 4096
N_CORES = 8
FSH = F // N_CORES      # 512 kw rows / vw cols per core
DSH = D // N_CORES      # 128 rw rows per core
LN_EPS = 1e-5

_STATE = {}


def _body(nc, tc, mybir, stage):
    f32 = mybir.dt.float32
    Alu = mybir.AluOpType
    Act = mybir.ActivationFunctionType
    AxX = mybir.AxisListType.X

    kw_d = nc.dram_tensor("kw_p", [128, 4096], f32, kind="ExternalInput").ap()
    vw_d = nc.dram_tensor("vw_p", [128, 4096], f32, kind="ExternalInput").ap()
    rw_d = nc.dram_tensor("rw_p", [128, 1024], f32, kind="ExternalInput").ap()
    # x, prev, tmk, tmr, lnw, lnb stacked: [8, 6*128], row j = vectors' d-slice j
    sm_d = nc.dram_tensor("smalls", [8, 768], f32, kind="ExternalInput").ap()

    xn_d = nc.dram_tensor("xn_out", [8, 128], f32, kind="ExternalOutput").ap()
    v_d = nc.dram_tensor("v_out", [8, 128], f32, kind="ExternalOutput").ap()
    r_d = nc.dram_tensor("r_out", [1, 128], f32, kind="ExternalOutput").ap()

    import contextlib
    with contextlib.ExitStack() as ctx:
        wp = ctx.enter_context(tc.tile_pool(name="w", bufs=1))
        vp = ctx.enter_context(tc.tile_pool(name="v", bufs=1))
        bp = ctx.enter_context(tc.tile_pool(name="bc", bufs=2, space="PSUM"))
        pp = ctx.enter_context(tc.tile_pool(name="ps", bufs=1, space="PSUM"))
        dp = ctx.enter_context(tc.tile_pool(name="dr", bufs=1, space="DRAM"))

        # ---- small packed DMA first, then bulk (same HWDGE FIFO: sm->kw->rw->vw)
        sm_sb = vp.tile([8, 768], f32, tag="sm")
        nc.sync.dma_start(out=sm_sb[:], in_=sm_d[:])
        x_row = sm_sb[:, 0:128]
        pv_row = sm_sb[:, 128:256]
        tk_row = sm_sb[:, 256:384]
        tr_row = sm_sb[:, 384:512]
        lw_row = sm_sb[:, 512:640]
        lb_row = sm_sb[:, 640:768]

        if stage >= 2:
            kw_sb = wp.tile([128, 4096], f32, tag="kw")
            rw_sb = wp.tile([128, 1024], f32, tag="rw")
            vw_sb = wp.tile([128, 4096], f32, tag="vw")
            for c in range(4):
                nc.sync.dma_start(out=kw_sb[:, c * 1024:(c + 1) * 1024],
                                  in_=kw_d[:, c * 1024:(c + 1) * 1024])
            nc.sync.dma_start(out=rw_sb[:], in_=rw_d[:])
            for c in range(4):
                nc.sync.dma_start(out=vw_sb[:, c * 1024:(c + 1) * 1024],
                                  in_=vw_d[:, c * 1024:(c + 1) * 1024])

        # ---- constants
        ones_c8 = vp.tile([8, 1], f32, tag="ones_c8")
        ones_r8 = vp.tile([1, 8], f32, tag="ones_r8")
        eps_t = vp.tile([1, 1], f32, tag="eps")
        eps8 = vp.tile([8, 1], f32, tag="eps8")
        nc.vector.memset(ones_c8[:], 1.0)
        nc.vector.memset(ones_r8[:], 1.0)
        nc.vector.memset(eps_t[:], LN_EPS)
        nc.vector.memset(eps8[:], LN_EPS)
        if stage >= 3:
            # one-hot row-selector matrices (lhsT for row-broadcast matmuls)
            sel8 = vp.tile([8, 1024], f32, tag="sel8")
            sel4 = vp.tile([4, 512], f32, tag="sel4")
            nc.gpsimd.memset(sel8[:], 0.0)
            nc.gpsimd.memset(sel4[:], 0.0)
            nc.gpsimd.affine_select(
                out=sel8[:].rearrange("p (j q) -> p j q", j=8),
                in_=sel8[:].rearrange("p (j q) -> p j q", j=8),
                compare_op=Alu.not_equal, fill=1.0, base=0,
                pattern=[[-1, 8], [0, 128]], channel_multiplier=1)
            nc.gpsimd.affine_select(
                out=sel4[:].rearrange("p (j q) -> p j q", j=4),
                in_=sel4[:].rearrange("p (j q) -> p j q", j=4),
                compare_op=Alu.not_equal, fill=1.0, base=0,
                pattern=[[-1, 4], [0, 128]], channel_multiplier=1)

        # ---- LayerNorm stats over 1024 elems laid out [8, 128]
        s2 = vp.tile([8, 2], f32, tag="s2")
        xsq = vp.tile([8, 128], f32, tag="xsq")
        nc.vector.tensor_reduce(out=s2[:, 0:1], in_=x_row, axis=AxX, op=Alu.add)
        nc.vector.scalar_tensor_tensor(out=xsq[:], in0=x_row, scalar=1.0,
                                       in1=x_row, op0=Alu.mult, op1=Alu.mult,
                                       accum_out=s2[:, 1:2])

        psum_s = pp.tile([1, 2], f32, tag="pmisc", bufs=2)
        nc.tensor.matmul(psum_s[:], ones_c8[:], s2[:], start=True, stop=True)
        ssum = vp.tile([1, 2], f32, tag="ssum")     # raw [sum_x, sum_x2]
        nc.scalar.copy(ssum[:], psum_s[:])
        psum_b = pp.tile([8, 2], f32, tag="pmisc", bufs=2)
        nc.tensor.matmul(psum_b[:], ones_r8[:], ssum[:], start=True, stop=True)
        bc8 = vp.tile([8, 2], f32, tag="bc8")       # per-part raw sums
        nc.scalar.mul(bc8[:], psum_b[:], 1.0 / D)   # [mean, E[x^2]] per part

        mean8 = bc8[:, 0:1]
        var8 = vp.tile([8, 1], f32, tag="var8")
        std8 = vp.tile([8, 1], f32, tag="std8")
        rstd8 = vp.tile([8, 1], f32, tag="rstd8")
        nc.vector.tensor_mul(var8[:], mean8, mean8)
        nc.vector.tensor_sub(var8[:], bc8[:, 1:2], var8[:])
        nc.scalar.activation(std8[:], var8[:], Act.Sqrt, bias=eps8[:])
        nc.vector.reciprocal(rstd8[:], std8[:])

        # Mix coefficients (xk = Ck*xn_pre + Ek with Ck = tmk*lnw,
        # Ek = tmk*lnb + prev*(1-tmk)) -- emitted here so the in-order DVE
        # runs them inside the PE/ACT stats round-trip idle window.
        b0k = vp.tile([8, 128], f32, tag="b0k")
        b0r = vp.tile([8, 128], f32, tag="b0r")
        ck = vp.tile([8, 128], f32, tag="ck")
        cr = vp.tile([8, 128], f32, tag="cr")
        ek = vp.tile([8, 128], f32, tag="ek")
        er = vp.tile([8, 128], f32, tag="er")
        nc.vector.tensor_mul(b0k[:], pv_row, tk_row)
        nc.vector.tensor_sub(b0k[:], pv_row, b0k[:])
        nc.vector.tensor_mul(b0r[:], pv_row, tr_row)
        nc.vector.tensor_sub(b0r[:], pv_row, b0r[:])
        nc.vector.tensor_mul(ck[:], tk_row, lw_row)
        nc.vector.tensor_mul(cr[:], tr_row, lw_row)
        nc.vector.tensor_mul(ek[:], tk_row, lb_row)
        nc.vector.tensor_add(ek[:], ek[:], b0k[:])
        nc.vector.tensor_mul(er[:], tr_row, lb_row)
        nc.vector.tensor_add(er[:], er[:], b0r[:])

        xn_pre = vp.tile([8, 128], f32, tag="xn_pre")
        nc.vector.tensor_scalar(out=xn_pre[:], in0=x_row,
                                scalar1=mean8, scalar2=rstd8[:],
                                op0=Alu.subtract, op1=Alu.mult)

        # ---- token mixes straight from xn_pre (critical path)
        xk_row = vp.tile([8, 128], f32, tag="xk")
        xr_row = vp.tile([8, 128], f32, tag="xr")
        nc.vector.tensor_mul(xk_row[:], xn_pre[:], ck[:])
        nc.vector.tensor_add(xk_row[:], xk_row[:], ek[:])
        nc.vector.tensor_mul(xr_row[:], xn_pre[:], cr[:])
        nc.vector.tensor_add(xr_row[:], xr_row[:], er[:])

        # full xn only feeds the output (off the critical path)
        xn_row = vp.tile([8, 128], f32, tag="xn")
        nc.vector.tensor_mul(xn_row[:], xn_pre[:], lw_row)
        nc.vector.tensor_add(xn_row[:], xn_row[:], lb_row)
        nc.sync.dma_start(out=xn_d[:], in_=xn_row[:])

        if stage < 3:
            return

        # ---- broadcast xk across partitions: [8,128] -> [128, 1024]
        xk_bc = vp.tile([128, 1024], f32, tag="xk_bc")
        for j in range(8):
            pb = bp.tile([128, 128], f32, tag="pb", name=f"pbk{j}")
            nc.tensor.matmul(pb[:], sel8[:, j * 128:(j + 1) * 128], xk_row[:],
                             start=True, stop=True)
            nc.scalar.copy(xk_bc[:, j * 128:(j + 1) * 128], pb[:])

        if stage < 4:
            return

        # ---- stage A: k chunk = sqrelu(kw_i @ xk), contraction split in
        #      halves so the first 4 dots overlap the 2nd half of xk_bc
        scratch = vp.tile([128, 1024], f32, tag="scratch")
        kh = vp.tile([128, 8], f32, tag="kh")       # [c, half] partials
        for h in range(2):
            for c in range(4):
                nc.vector.scalar_tensor_tensor(
                    out=scratch[:, h * 512:(h + 1) * 512],
                    in0=kw_sb[:, c * 1024 + h * 512: c * 1024 + (h + 1) * 512],
                    scalar=1.0, in1=xk_bc[:, h * 512:(h + 1) * 512],
                    op0=Alu.mult, op1=Alu.mult,
                    accum_out=kh[:, 2 * c + h: 2 * c + h + 1])
        k_sb = vp.tile([128, 4], f32, tag="k")
        nc.vector.tensor_add(k_sb[:], kh[:].rearrange("p (c h) -> p c h", h=2)[:, :, 0],
                             kh[:].rearrange("p (c h) -> p c h", h=2)[:, :, 1])
        krelu = vp.tile([128, 4], f32, tag="krelu")
        ksq = vp.tile([128, 4], f32, tag="ksq")
        nc.vector.tensor_scalar_max(krelu[:], k_sb[:], 0.0)
        nc.vector.tensor_mul(ksq[:], krelu[:], krelu[:])

        if stage < 5:
            return

        # ---- broadcast xr (during kw dots) and compute r
        xr_bc = vp.tile([128, 1024], f32, tag="xr_bc")
        for j in range(8):
            pb = bp.tile([128, 128], f32, tag="pb", name=f"pbr{j}")
            nc.tensor.matmul(pb[:], sel8[:, j * 128:(j + 1) * 128], xr_row[:],
                             start=True, stop=True)
            nc.scalar.copy(xr_bc[:, j * 128:(j + 1) * 128], pb[:])

        pre_r = vp.tile([128, 1], f32, tag="pre_r")
        nc.vector.scalar_tensor_tensor(
            out=scratch[:], in0=rw_sb[:], scalar=1.0, in1=xr_bc[:],
            op0=Alu.mult, op1=Alu.mult, accum_out=pre_r[:])
        r_sb = vp.tile([128, 1], f32, tag="r")
        nc.scalar.activation(r_sb[:], pre_r[:], Act.Sigmoid)

        if stage < 6:
            return

        # ---- k broadcast: PE transpose, then 4 selector matmuls into one
        #      PSUM bank (vw dots read it straight from PSUM)
        from concourse.masks import make_identity
        ident = vp.tile([128, 128], f32, tag="ident")
        make_identity(nc, ident)
        kT_ps = pp.tile([4, 128], f32, tag="pmisc", bufs=2)
        nc.tensor.transpose(kT_ps[:], ksq[:], ident[:])
        kT = vp.tile([4, 128], f32, tag="kT")
        nc.scalar.copy(kT[:], kT_ps[:])
        k_bc = pp.tile([128, 512], f32, tag="kbc_ps", bufs=1)
        for c in range(4):
            nc.tensor.matmul(k_bc[:, c * 128:(c + 1) * 128],
                             sel4[:, c * 128:(c + 1) * 128], kT[:],
                             start=True, stop=True)

        # ---- stage V: v partial, 8 d-chunks of [128, 512] x k_bc
        v_sb = vp.tile([128, 8], f32, tag="v")
        for m in range(8):
            nc.vector.scalar_tensor_tensor(
                out=scratch[:, 0:512], in0=vw_sb[:, m * 512:(m + 1) * 512],
                scalar=1.0, in1=k_bc[:],
                op0=Alu.mult, op1=Alu.mult, accum_out=v_sb[:, m:m + 1])

        # ---- outputs in row form (contiguous DMA): transpose via PE
        vT_ps = pp.tile([8, 128], f32, tag="pmisc", bufs=2)
        nc.tensor.transpose(vT_ps[:], v_sb[:], ident[:])
        vT = vp.tile([8, 128], f32, tag="vT")
        nc.scalar.copy(vT[:], vT_ps[:])
        nc.sync.dma_start(out=v_d[:], in_=vT[:])

        rT_ps = pp.tile([1, 128], f32, tag="pmisc", bufs=2)
        nc.tensor.transpose(rT_ps[:], r_sb[:], ident[:])
        rT = vp.tile([1, 128], f32, tag="rT")
        nc.scalar.copy(rT[:], rT_ps[:])
        nc.sync.dma_start(out=r_d[:], in_=rT[:])


def _build(stage=6):
    import concourse.bacc as bacc
    import concourse.tile as tile
    from concourse import mybir

    nc = bacc.Bacc("TRN2", target_bir_lowering=False, debug=False,
                   num_devices=N_CORES)
    with tile.TileContext(nc) as tc:
        _body(nc, tc, mybir, stage)
    nc.compile()
    return nc


def _prep_shared(kw, vw, rw):
    """Slice + reshape weights per core (rows onto 128 partitions)."""
    kw_p, vw_p, rw_p = [], [], []
    for i in range(N_CORES):
        A = kw[i * FSH:(i + 1) * FSH, :]                # (512, 1024) rows f
        A = A.reshape(4, 128, 1024).transpose(1, 0, 2)  # [p, c, d]
        kw_p.append(np.ascontiguousarray(A.reshape(128, 4096)))

        B = rw[i * DSH:(i + 1) * DSH, :]                # (128, 1024) rows d
        rw_p.append(np.ascontiguousarray(B))

        C = vw[:, i * FSH:(i + 1) * FSH]                # (1024, 512) rows d
        C = C.reshape(8, 128, FSH).transpose(1, 0, 2)   # [p, m, f]
        vw_p.append(np.ascontiguousarray(C.reshape(128, 4096)))
    return kw_p, vw_p, rw_p


def _prep_smalls(x, state, tmk, tmr, lnw, lnb):
    sm = np.stack([x.reshape(8, 128), state[0].reshape(8, 128),
                   tmk.reshape(8, 128), tmr.reshape(8, 128),
                   lnw.reshape(8, 128), lnb.reshape(8, 128)], axis=1)
    return np.ascontiguousarray(sm.reshape(8, 768))


def kernel(x, state, time_mix_k, time_mix_r, kw, vw, rw, ln_weight, ln_bias):
    from concourse import bass_utils

    x = np.asarray(x, dtype=np.float32)
    state = np.asarray(state, dtype=np.float32)
    kw = np.asarray(kw, dtype=np.float32)
    vw = np.asarray(vw, dtype=np.float32)
    rw = np.asarray(rw, dtype=np.float32)
    tmk = np.asarray(time_mix_k, dtype=np.float32)
    tmr = np.asarray(time_mix_r, dtype=np.float32)
    lnw = np.asarray(ln_weight, dtype=np.float32)
    lnb = np.asarray(ln_bias, dtype=np.float32)

    if "nc" not in _STATE:
        _STATE["nc"] = _build()
    nc = _STATE["nc"]

    kw_p, vw_p, rw_p = _prep_shared(kw, vw, rw)
    sm = _prep_smalls(x, state, tmk, tmr, lnw, lnb)

    in_maps = [{"kw_p": kw_p[i], "vw_p": vw_p[i], "rw_p": rw_p[i], "smalls": sm}
               for i in range(N_CORES)]

    res = bass_utils.run_bass_kernel_spmd(nc, in_maps, core_ids=list(range(N_CORES)))

    # unshard: v = sum of partials, r = concat of chunks
    v = np.zeros(D, dtype=np.float64)
    for i in range(N_CORES):
        v += res.results[i]["v_out"].reshape(D).astype(np.float64)
    r = np.concatenate([res.results[i]["r_out"].reshape(DSH)
                        for i in range(N_CORES)])
    out = x + r * v.astype(np.float32)
    xn = res.results[0]["xn_out"].reshape(D)
    return np.asarray(out, dtype=np.float32), np.asarray(xn, dtype=np.float32)


# revision 14
# speedup vs baseline: 1.1430x; 1.0146x over previous
"""RWKV ChannelMixer (single-token) on 8 Trainium2 NeuronCores.

Math (reference):
    xn  = LayerNorm(x) * ln_w + ln_b
    xk  = xn*tmk + prev*(1-tmk);  xr = xn*tmr + prev*(1-tmr)
    r   = sigmoid(rw @ xr)                       # (D,)
    k   = relu(kw @ xk)^2                        # (F,)
    out = x + r * (vw @ k)                       # (D,)
    returns (out, xn)

Sharding (8 cores, no collectives -- cross-core sync costs ~60us here):
    kw: F-row-sharded (512 rows/core)  -> local k chunk (512,)
    vw: F-col-sharded (512 cols/core)  -> partial v_i = vw[:,Fi] @ k_i (1024,)
    rw: D-row-sharded (128 rows/core)  -> r chunk (128,)
    LN/mix replicated.  Host unshard: v = sum_i v_i, r = concat(r_i),
    out = x + r*v.

Engines: dot-products run on the Vector engine (tensor_tensor_reduce,
fp32 @ ~1 elem/lane/cycle ~ 490GB/s > 358GB/s HBM/core, so the kernel
stays DMA-bound).  TensorE only does tiny selector-matmul broadcasts /
transposes.  Weight matrices stream through SBUF in natural row-major
layout (host reshapes rows onto 128 partitions; no transposes).
"""

import sys
import numpy as np

for _p in ("/opt/trn_rl_repo", "/root/.axon_site/_ro/trn_rl_repo"):
    if _p not in sys.path:
        sys.path.append(_p)

D = 1024
F = 4096
N_CORES = 8
FSH = F // N_CORES      # 512 kw rows / vw cols per core
DSH = D // N_CORES      # 128 rw rows per core
LN_EPS = 1e-5

_STATE = {}


def _body(nc, tc, mybir, stage):
    f32 = mybir.dt.float32
    Alu = mybir.AluOpType
    Act = mybir.ActivationFunctionType
    AxX = mybir.AxisListType.X

    kw_d = nc.dram_tensor("kw_p", [128, 4096], f32, kind="ExternalInput").ap()
    vw_d = nc.dram_tensor("vw_p", [128, 4096], f32, kind="ExternalInput").ap()
    rw_d = nc.dram_tensor("rw_p", [128, 1024], f32, kind="ExternalInput").ap()
    # x, prev, tmk, tmr, lnw, lnb stacked: [8, 6*128], row j = vectors' d-slice j
    sm_d = nc.dram_tensor("smalls", [8, 768], f32, kind="ExternalInput").ap()

    xn_d = nc.dram_tensor("xn_out", [8, 128], f32, kind="ExternalOutput").ap()
    vr_d = nc.dram_tensor("vr_out", [9, 128], f32, kind="ExternalOutput").ap()

    import contextlib
    with contextlib.ExitStack() as ctx:
        wp = ctx.enter_context(tc.tile_pool(name="w", bufs=1))
        vp = ctx.enter_context(tc.tile_pool(name="v", bufs=1))
        bp = ctx.enter_context(tc.tile_pool(name="bc", bufs=2, space="PSUM"))
        pp = ctx.enter_context(tc.tile_pool(name="ps", bufs=1, space="PSUM"))
        dp = ctx.enter_context(tc.tile_pool(name="dr", bufs=1, space="DRAM"))

        # ---- small packed DMA first, then bulk (same HWDGE FIFO: sm->kw->rw->vw)
        sm_sb = vp.tile([8, 768], f32, tag="sm")
        nc.sync.dma_start(out=sm_sb[:], in_=sm_d[:])
        x_row = sm_sb[:, 0:128]
        pv_row = sm_sb[:, 128:256]
        tk_row = sm_sb[:, 256:384]
        tr_row = sm_sb[:, 384:512]
        lw_row = sm_sb[:, 512:640]
        lb_row = sm_sb[:, 640:768]

        if stage >= 2:
            kw_sb = wp.tile([128, 4096], f32, tag="kw")
            rw_sb = wp.tile([128, 1024], f32, tag="rw")
            vw_sb = wp.tile([128, 4096], f32, tag="vw")
            for c in range(4):
                nc.sync.dma_start(out=kw_sb[:, c * 1024:(c + 1) * 1024],
                                  in_=kw_d[:, c * 1024:(c + 1) * 1024])
            nc.sync.dma_start(out=rw_sb[:], in_=rw_d[:])
            for c in range(4):
                nc.sync.dma_start(out=vw_sb[:, c * 1024:(c + 1) * 1024],
                                  in_=vw_d[:, c * 1024:(c + 1) * 1024])

        # ---- constants
        ones_c8 = vp.tile([8, 1], f32, tag="ones_c8")
        ones_r8 = vp.tile([1, 8], f32, tag="ones_r8")
        eps_t = vp.tile([1, 1], f32, tag="eps")
        eps8 = vp.tile([8, 1], f32, tag="eps8")
        nc.vector.memset(ones_c8[:], 1.0)
        nc.vector.memset(ones_r8[:], 1.0)
        nc.vector.memset(eps_t[:], LN_EPS)
        nc.vector.memset(eps8[:], LN_EPS)
        if stage >= 3:
            # one-hot row-selector matrices (lhsT for row-broadcast matmuls)
            sel8 = vp.tile([8, 1024], f32, tag="sel8")
            sel4 = vp.tile([4, 512], f32, tag="sel4")
            nc.gpsimd.memset(sel8[:], 0.0)
            nc.gpsimd.memset(sel4[:], 0.0)
            nc.gpsimd.affine_select(
                out=sel8[:].rearrange("p (j q) -> p j q", j=8),
                in_=sel8[:].rearrange("p (j q) -> p j q", j=8),
                compare_op=Alu.not_equal, fill=1.0, base=0,
                pattern=[[-1, 8], [0, 128]], channel_multiplier=1)
            nc.gpsimd.affine_select(
                out=sel4[:].rearrange("p (j q) -> p j q", j=4),
                in_=sel4[:].rearrange("p (j q) -> p j q", j=4),
                compare_op=Alu.not_equal, fill=1.0, base=0,
                pattern=[[-1, 4], [0, 128]], channel_multiplier=1)

        # ---- LayerNorm stats over 1024 elems laid out [8, 128]
        s2 = vp.tile([8, 2], f32, tag="s2")
        xsq = vp.tile([8, 128], f32, tag="xsq")
        nc.vector.tensor_reduce(out=s2[:, 0:1], in_=x_row, axis=AxX, op=Alu.add)
        nc.vector.scalar_tensor_tensor(out=xsq[:], in0=x_row, scalar=1.0,
                                       in1=x_row, op0=Alu.mult, op1=Alu.mult,
                                       accum_out=s2[:, 1:2])

        psum_s = pp.tile([1, 2], f32, tag="pmisc", bufs=2)
        nc.tensor.matmul(psum_s[:], ones_c8[:], s2[:], start=True, stop=True)
        ssum = vp.tile([1, 2], f32, tag="ssum")     # raw [sum_x, sum_x2]
        nc.scalar.copy(ssum[:], psum_s[:])
        psum_b = pp.tile([8, 2], f32, tag="pmisc", bufs=2)
        nc.tensor.matmul(psum_b[:], ones_r8[:], ssum[:], start=True, stop=True)
        bc8 = vp.tile([8, 2], f32, tag="bc8")       # per-part raw sums
        nc.scalar.mul(bc8[:], psum_b[:], 1.0 / D)   # [mean, E[x^2]] per part

        mean8 = bc8[:, 0:1]
        var8 = vp.tile([8, 1], f32, tag="var8")
        std8 = vp.tile([8, 1], f32, tag="std8")
        rstd8 = vp.tile([8, 1], f32, tag="rstd8")
        nc.vector.tensor_mul(var8[:], mean8, mean8)
        nc.vector.tensor_sub(var8[:], bc8[:, 1:2], var8[:])
        nc.scalar.activation(std8[:], var8[:], Act.Sqrt, bias=eps8[:])
        nc.vector.reciprocal(rstd8[:], std8[:])

        # Mix coefficients (xk = Ck*xn_pre + Ek with Ck = tmk*lnw,
        # Ek = tmk*lnb + prev*(1-tmk)) -- emitted here so the in-order DVE
        # runs them inside the PE/ACT stats round-trip idle window.
        b0k = vp.tile([8, 128], f32, tag="b0k")
        b0r = vp.tile([8, 128], f32, tag="b0r")
        ck = vp.tile([8, 128], f32, tag="ck")
        cr = vp.tile([8, 128], f32, tag="cr")
        ek = vp.tile([8, 128], f32, tag="ek")
        er = vp.tile([8, 128], f32, tag="er")
        nc.vector.tensor_mul(b0k[:], pv_row, tk_row)
        nc.vector.tensor_sub(b0k[:], pv_row, b0k[:])
        nc.vector.tensor_mul(b0r[:], pv_row, tr_row)
        nc.vector.tensor_sub(b0r[:], pv_row, b0r[:])
        nc.vector.tensor_mul(ck[:], tk_row, lw_row)
        nc.vector.tensor_mul(cr[:], tr_row, lw_row)
        nc.vector.tensor_mul(ek[:], tk_row, lb_row)
        nc.vector.tensor_add(ek[:], ek[:], b0k[:])
        nc.vector.tensor_mul(er[:], tr_row, lb_row)
        nc.vector.tensor_add(er[:], er[:], b0r[:])

        xn_pre = vp.tile([8, 128], f32, tag="xn_pre")
        nc.vector.tensor_scalar(out=xn_pre[:], in0=x_row,
                                scalar1=mean8, scalar2=rstd8[:],
                                op0=Alu.subtract, op1=Alu.mult)

        # ---- token mixes straight from xn_pre (critical path)
        xk_row = vp.tile([8, 128], f32, tag="xk")
        xr_row = vp.tile([8, 128], f32, tag="xr")
        nc.vector.tensor_mul(xk_row[:], xn_pre[:], ck[:])
        nc.vector.tensor_add(xk_row[:], xk_row[:], ek[:])
        nc.vector.tensor_mul(xr_row[:], xn_pre[:], cr[:])
        nc.vector.tensor_add(xr_row[:], xr_row[:], er[:])

        # full xn only feeds the output (off the critical path)
        xn_row = vp.tile([8, 128], f32, tag="xn")
        nc.vector.tensor_mul(xn_row[:], xn_pre[:], lw_row)
        nc.vector.tensor_add(xn_row[:], xn_row[:], lb_row)
        nc.sync.dma_start(out=xn_d[:], in_=xn_row[:])

        if stage < 3:
            return

        # ---- broadcast xk across partitions: [8,128] -> [128, 1024]
        xk_bc = vp.tile([128, 1024], f32, tag="xk_bc")
        for j in range(8):
            pb = bp.tile([128, 128], f32, tag="pb", name=f"pbk{j}")
            nc.tensor.matmul(pb[:], sel8[:, j * 128:(j + 1) * 128], xk_row[:],
                             start=True, stop=True)
            nc.scalar.copy(xk_bc[:, j * 128:(j + 1) * 128], pb[:])

        if stage < 4:
            return

        # ---- stage A: k chunk = sqrelu(kw_i @ xk), contraction split in
        #      halves so the first 4 dots overlap the 2nd half of xk_bc
        scratch = vp.tile([128, 1024], f32, tag="scratch")
        kh = vp.tile([128, 8], f32, tag="kh")       # [c, half] partials
        for h in range(2):
            for c in range(4):
                nc.vector.scalar_tensor_tensor(
                    out=scratch[:, h * 512:(h + 1) * 512],
                    in0=kw_sb[:, c * 1024 + h * 512: c * 1024 + (h + 1) * 512],
                    scalar=1.0, in1=xk_bc[:, h * 512:(h + 1) * 512],
                    op0=Alu.mult, op1=Alu.mult,
                    accum_out=kh[:, 2 * c + h: 2 * c + h + 1])
        k_sb = vp.tile([128, 4], f32, tag="k")
        nc.vector.tensor_add(k_sb[:], kh[:].rearrange("p (c h) -> p c h", h=2)[:, :, 0],
                             kh[:].rearrange("p (c h) -> p c h", h=2)[:, :, 1])
        vr_sb = vp.tile([128, 9], f32, tag="vr")
        krelu = vp.tile([128, 4], f32, tag="krelu")
        ksq = vp.tile([128, 4], f32, tag="ksq")
        nc.vector.tensor_scalar_max(krelu[:], k_sb[:], 0.0)
        nc.vector.tensor_mul(ksq[:], krelu[:], krelu[:])

        if stage < 5:
            return

        # ---- k broadcast: PE transpose, then 4 selector matmuls into one
        #      PSUM bank (vw dots read it straight from PSUM)
        from concourse.masks import make_identity
        ident = vp.tile([128, 128], f32, tag="ident")
        make_identity(nc, ident)
        kT_ps = pp.tile([4, 128], f32, tag="pmisc", bufs=2)
        nc.tensor.transpose(kT_ps[:], ksq[:], ident[:])
        kT = vp.tile([4, 128], f32, tag="kT")
        nc.scalar.copy(kT[:], kT_ps[:])
        k_bc = pp.tile([128, 512], f32, tag="kbc_ps", bufs=1)
        for c in range(4):
            nc.tensor.matmul(k_bc[:, c * 128:(c + 1) * 128],
                             sel4[:, c * 128:(c + 1) * 128], kT[:],
                             start=True, stop=True)
        if stage < 6:
            return

        # ---- broadcast xr (during kw dots) and compute r
        xr_bc = vp.tile([128, 1024], f32, tag="xr_bc")
        for j in range(8):
            pb = bp.tile([128, 128], f32, tag="pb", name=f"pbr{j}")
            nc.tensor.matmul(pb[:], sel8[:, j * 128:(j + 1) * 128], xr_row[:],
                             start=True, stop=True)
            nc.scalar.copy(xr_bc[:, j * 128:(j + 1) * 128], pb[:])

        pre_r = vp.tile([128, 1], f32, tag="pre_r")
        nc.vector.scalar_tensor_tensor(
            out=scratch[:], in0=rw_sb[:], scalar=1.0, in1=xr_bc[:],
            op0=Alu.mult, op1=Alu.mult, accum_out=pre_r[:])
        nc.scalar.activation(vr_sb[:, 8:9], pre_r[:], Act.Sigmoid)


        # ---- stage V: v partial, 8 d-chunks of [128, 512] x k_bc
        for m in range(8):
            nc.vector.scalar_tensor_tensor(
                out=scratch[:, 0:512], in0=vw_sb[:, m * 512:(m + 1) * 512],
                scalar=1.0, in1=k_bc[:],
                op0=Alu.mult, op1=Alu.mult, accum_out=vr_sb[:, m:m + 1])

        # ---- outputs in row form (contiguous DMA): one transpose via PE
        vrT_ps = pp.tile([9, 128], f32, tag="pmisc", bufs=2)
        nc.tensor.transpose(vrT_ps[:], vr_sb[:], ident[:])
        vrT = vp.tile([9, 128], f32, tag="vrT")
        nc.scalar.copy(vrT[:], vrT_ps[:])
        nc.sync.dma_start(out=vr_d[:], in_=vrT[:])


def _build(stage=6):
    import concourse.bacc as bacc
    import concourse.tile as tile
    from concourse import mybir

    nc = bacc.Bacc("TRN2", target_bir_lowering=False, debug=False,
                   num_devices=N_CORES)
    with tile.TileContext(nc) as tc:
        _body(nc, tc, mybir, stage)
    nc.compile()
    return nc


def _prep_shared(kw, vw, rw):
    """Slice + reshape weights per core (rows onto 128 partitions)."""
    kw_p, vw_p, rw_p = [], [], []
    for i in range(N_CORES):
        A = kw[i * FSH:(i + 1) * FSH, :]                # (512, 1024) rows f
        A = A.reshape(4, 128, 1024).transpose(1, 0, 2)  # [p, c, d]
        kw_p.append(np.ascontiguousarray(A.reshape(128, 4096)))

        B = rw[i * DSH:(i + 1) * DSH, :]                # (128, 1024) rows d
        rw_p.append(np.ascontiguousarray(B))

        C = vw[:, i * FSH:(i + 1) * FSH]                # (1024, 512) rows d
        C = C.reshape(8, 128, FSH).transpose(1, 0, 2)   # [p, m, f]
        vw_p.append(np.ascontiguousarray(C.reshape(128, 4096)))
    return kw_p, vw_p, rw_p


def _prep_smalls(x, state, tmk, tmr, lnw, lnb):
    sm = np.stack([x.reshape(8, 128), state[0].reshape(8, 128),
                   tmk.reshape(8, 128), tmr.reshape(8, 128),
                   lnw.reshape(8, 128), lnb.reshape(8, 128)], axis=1)
    return np.ascontiguousarray(sm.reshape(8, 768))


def kernel(x, state, time_mix_k, time_mix_r, kw, vw, rw, ln_weight, ln_bias):
    from concourse import bass_utils

    x = np.asarray(x, dtype=np.float32)
    state = np.asarray(state, dtype=np.float32)
    kw = np.asarray(kw, dtype=np.float32)
    vw = np.asarray(vw, dtype=np.float32)
    rw = np.asarray(rw, dtype=np.float32)
    tmk = np.asarray(time_mix_k, dtype=np.float32)
    tmr = np.asarray(time_mix_r, dtype=np.float32)
    lnw = np.asarray(ln_weight, dtype=np.float32)
    lnb = np.asarray(ln_bias, dtype=np.float32)

    if "nc" not in _STATE:
        _STATE["nc"] = _build()
    nc = _STATE["nc"]

    kw_p, vw_p, rw_p = _prep_shared(kw, vw, rw)
    sm = _prep_smalls(x, state, tmk, tmr, lnw, lnb)

    in_maps = [{"kw_p": kw_p[i], "vw_p": vw_p[i], "rw_p": rw_p[i], "smalls": sm}
               for i in range(N_CORES)]

    res = bass_utils.run_bass_kernel_spmd(nc, in_maps, core_ids=list(range(N_CORES)))

    # unshard: v = sum of partials, r = concat of chunks
    v = np.zeros(D, dtype=np.float64)
    for i in range(N_CORES):
        v += res.results[i]["vr_out"][:8].reshape(D).astype(np.float64)
    r = np.concatenate([res.results[i]["vr_out"][8]
                        for i in range(N_CORES)])
    out = x + r * v.astype(np.float32)
    xn = res.results[0]["xn_out"].reshape(D)
    return np.asarray(out, dtype=np.float32), np.asarray(xn, dtype=np.float32)


# revision 15
# speedup vs baseline: 1.2403x; 1.0852x over previous
"""RWKV ChannelMixer (single-token) on 8 Trainium2 NeuronCores.

Math (reference):
    xn  = LayerNorm(x) * ln_w + ln_b
    xk  = xn*tmk + prev*(1-tmk);  xr = xn*tmr + prev*(1-tmr)
    r   = sigmoid(rw @ xr)                       # (D,)
    k   = relu(kw @ xk)^2                        # (F,)
    out = x + r * (vw @ k)                       # (D,)
    returns (out, xn)

Sharding (8 cores, no collectives -- cross-core sync costs ~60us here):
    kw: F-row-sharded (512 rows/core)  -> local k chunk (512,)
    vw: F-col-sharded (512 cols/core)  -> partial v_i = vw[:,Fi] @ k_i (1024,)
    rw: D-row-sharded (128 rows/core)  -> r chunk (128,)
    LN/mix replicated.  Host unshard: v = sum_i v_i, r = concat(r_i),
    out = x + r*v.

Engines: dot-products run on the Vector engine (tensor_tensor_reduce,
fp32 @ ~1 elem/lane/cycle ~ 490GB/s > 358GB/s HBM/core, so the kernel
stays DMA-bound).  TensorE only does tiny selector-matmul broadcasts /
transposes.  Weight matrices stream through SBUF in natural row-major
layout (host reshapes rows onto 128 partitions; no transposes).
"""

import sys
import numpy as np

for _p in ("/opt/trn_rl_repo", "/root/.axon_site/_ro/trn_rl_repo"):
    if _p not in sys.path:
        sys.path.append(_p)

D = 1024
F = 4096
N_CORES = 8
FSH = F // N_CORES      # 512 kw rows / vw cols per core
DSH = D // N_CORES      # 128 rw rows per core
LN_EPS = 1e-5

_STATE = {}


def _body(nc, tc, mybir, stage):
    f32 = mybir.dt.float32
    Alu = mybir.AluOpType
    Act = mybir.ActivationFunctionType
    AxX = mybir.AxisListType.X

    kw_d = nc.dram_tensor("kw_p", [128, 4096], f32, kind="ExternalInput").ap()
    vw_d = nc.dram_tensor("vw_p", [128, 4096], f32, kind="ExternalInput").ap()
    rw_d = nc.dram_tensor("rw_p", [128, 1024], f32, kind="ExternalInput").ap()
    # x, prev, tmk, tmr, lnw, lnb stacked: [8, 6*128], row j = vectors' d-slice j
    sm_d = nc.dram_tensor("smalls", [8, 768], f32, kind="ExternalInput").ap()

    xn_d = nc.dram_tensor("xn_out", [8, 128], f32, kind="ExternalOutput").ap()
    vr_d = nc.dram_tensor("vr_out", [9, 128], f32, kind="ExternalOutput").ap()

    import contextlib
    with contextlib.ExitStack() as ctx:
        wp = ctx.enter_context(tc.tile_pool(name="w", bufs=1))
        vp = ctx.enter_context(tc.tile_pool(name="v", bufs=1))
        bp = ctx.enter_context(tc.tile_pool(name="bc", bufs=2, space="PSUM"))
        pp = ctx.enter_context(tc.tile_pool(name="ps", bufs=1, space="PSUM"))
        dp = ctx.enter_context(tc.tile_pool(name="dr", bufs=1, space="DRAM"))

        # ---- small packed DMA first, then bulk (same HWDGE FIFO: sm->kw->rw->vw)
        sm_sb = vp.tile([8, 768], f32, tag="sm")
        nc.sync.dma_start(out=sm_sb[:], in_=sm_d[:])
        x_row = sm_sb[:, 0:128]
        pv_row = sm_sb[:, 128:256]
        tk_row = sm_sb[:, 256:384]
        tr_row = sm_sb[:, 384:512]
        lw_row = sm_sb[:, 512:640]
        lb_row = sm_sb[:, 640:768]

        if stage >= 2:
            kw_sb = wp.tile([128, 4096], f32, tag="kw")
            rw_sb = wp.tile([128, 1024], f32, tag="rw")
            vw_sb = wp.tile([128, 4096], f32, tag="vw")
            for c in range(4):
                nc.sync.dma_start(out=kw_sb[:, c * 1024:(c + 1) * 1024],
                                  in_=kw_d[:, c * 1024:(c + 1) * 1024])
            nc.sync.dma_start(out=rw_sb[:], in_=rw_d[:])
            for c in range(4):
                nc.sync.dma_start(out=vw_sb[:, c * 1024:(c + 1) * 1024],
                                  in_=vw_d[:, c * 1024:(c + 1) * 1024])

        # ---- constants
        ones_c8 = vp.tile([8, 1], f32, tag="ones_c8")
        ones_r8 = vp.tile([1, 8], f32, tag="ones_r8")
        eps_t = vp.tile([1, 1], f32, tag="eps")
        eps8 = vp.tile([8, 1], f32, tag="eps8")
        nc.vector.memset(ones_c8[:], 1.0)
        nc.vector.memset(ones_r8[:], 1.0)
        nc.vector.memset(eps_t[:], LN_EPS)
        nc.vector.memset(eps8[:], LN_EPS)
        warm_sqrt = vp.tile([1, 1], f32, tag="warm_sqrt")
        nc.scalar.activation(warm_sqrt[:], eps_t[:], Act.Sqrt)
        if stage >= 3:
            # one-hot row-selector matrices (lhsT for row-broadcast matmuls)
            sel8 = vp.tile([8, 1024], f32, tag="sel8")
            sel4 = vp.tile([4, 512], f32, tag="sel4")
            nc.gpsimd.memset(sel8[:], 0.0)
            nc.gpsimd.memset(sel4[:], 0.0)
            nc.gpsimd.affine_select(
                out=sel8[:].rearrange("p (j q) -> p j q", j=8),
                in_=sel8[:].rearrange("p (j q) -> p j q", j=8),
                compare_op=Alu.not_equal, fill=1.0, base=0,
                pattern=[[-1, 8], [0, 128]], channel_multiplier=1)
            nc.gpsimd.affine_select(
                out=sel4[:].rearrange("p (j q) -> p j q", j=4),
                in_=sel4[:].rearrange("p (j q) -> p j q", j=4),
                compare_op=Alu.not_equal, fill=1.0, base=0,
                pattern=[[-1, 4], [0, 128]], channel_multiplier=1)

        # ---- LayerNorm stats over 1024 elems laid out [8, 128]
        s2 = vp.tile([8, 2], f32, tag="s2")
        xsq = vp.tile([8, 128], f32, tag="xsq")
        nc.vector.tensor_reduce(out=s2[:, 0:1], in_=x_row, axis=AxX, op=Alu.add)
        nc.vector.scalar_tensor_tensor(out=xsq[:], in0=x_row, scalar=1.0,
                                       in1=x_row, op0=Alu.mult, op1=Alu.mult,
                                       accum_out=s2[:, 1:2])

        psum_s = pp.tile([1, 2], f32, tag="pmisc", bufs=2)
        nc.tensor.matmul(psum_s[:], ones_c8[:], s2[:], start=True, stop=True)
        ssum = vp.tile([1, 2], f32, tag="ssum")     # raw [sum_x, sum_x2]
        nc.scalar.copy(ssum[:], psum_s[:])
        psum_b = pp.tile([8, 2], f32, tag="pmisc", bufs=2)
        nc.tensor.matmul(psum_b[:], ones_r8[:], ssum[:], start=True, stop=True)
        bc8 = vp.tile([8, 2], f32, tag="bc8")       # per-part raw sums
        nc.scalar.mul(bc8[:], psum_b[:], 1.0 / D)   # [mean, E[x^2]] per part

        mean8 = bc8[:, 0:1]
        var8 = vp.tile([8, 1], f32, tag="var8")
        std8 = vp.tile([8, 1], f32, tag="std8")
        rstd8 = vp.tile([8, 1], f32, tag="rstd8")
        nc.vector.tensor_mul(var8[:], mean8, mean8)
        nc.vector.tensor_sub(var8[:], bc8[:, 1:2], var8[:])
        nc.scalar.activation(std8[:], var8[:], Act.Sqrt, bias=eps8[:])
        nc.vector.reciprocal(rstd8[:], std8[:])

        # Mix coefficients (xk = Ck*xn_pre + Ek with Ck = tmk*lnw,
        # Ek = tmk*lnb + prev*(1-tmk)) -- emitted here so the in-order DVE
        # runs them inside the PE/ACT stats round-trip idle window.
        b0k = vp.tile([8, 128], f32, tag="b0k")
        b0r = vp.tile([8, 128], f32, tag="b0r")
        ck = vp.tile([8, 128], f32, tag="ck")
        cr = vp.tile([8, 128], f32, tag="cr")
        ek = vp.tile([8, 128], f32, tag="ek")
        er = vp.tile([8, 128], f32, tag="er")
        nc.vector.tensor_mul(b0k[:], pv_row, tk_row)
        nc.vector.tensor_sub(b0k[:], pv_row, b0k[:])
        nc.vector.tensor_mul(b0r[:], pv_row, tr_row)
        nc.vector.tensor_sub(b0r[:], pv_row, b0r[:])
        nc.vector.tensor_mul(ck[:], tk_row, lw_row)
        nc.vector.tensor_mul(cr[:], tr_row, lw_row)
        nc.vector.tensor_mul(ek[:], tk_row, lb_row)
        nc.vector.tensor_add(ek[:], ek[:], b0k[:])
        nc.vector.tensor_mul(er[:], tr_row, lb_row)
        nc.vector.tensor_add(er[:], er[:], b0r[:])

        xn_pre = vp.tile([8, 128], f32, tag="xn_pre")
        nc.vector.tensor_scalar(out=xn_pre[:], in0=x_row,
                                scalar1=mean8, scalar2=rstd8[:],
                                op0=Alu.subtract, op1=Alu.mult)

        # ---- token mixes straight from xn_pre (critical path)
        xk_row = vp.tile([8, 128], f32, tag="xk")
        xr_row = vp.tile([8, 128], f32, tag="xr")
        nc.vector.tensor_mul(xk_row[:], xn_pre[:], ck[:])
        nc.vector.tensor_add(xk_row[:], xk_row[:], ek[:])
        nc.vector.tensor_mul(xr_row[:], xn_pre[:], cr[:])
        nc.vector.tensor_add(xr_row[:], xr_row[:], er[:])

        # full xn only feeds the output (off the critical path)
        xn_row = vp.tile([8, 128], f32, tag="xn")
        nc.vector.tensor_mul(xn_row[:], xn_pre[:], lw_row)
        nc.vector.tensor_add(xn_row[:], xn_row[:], lb_row)
        nc.sync.dma_start(out=xn_d[:], in_=xn_row[:])

        if stage < 3:
            return

        # ---- broadcast xk across partitions: [8,128] -> [128, 1024]
        xk_bc = vp.tile([128, 1024], f32, tag="xk_bc")
        for j in range(8):
            pb = bp.tile([128, 128], f32, tag="pb", name=f"pbk{j}")
            nc.tensor.matmul(pb[:], sel8[:, j * 128:(j + 1) * 128], xk_row[:],
                             start=True, stop=True)
            nc.scalar.copy(xk_bc[:, j * 128:(j + 1) * 128], pb[:])

        if stage < 4:
            return

        # ---- stage A: k chunk = sqrelu(kw_i @ xk), contraction split in
        #      halves so the first 4 dots overlap the 2nd half of xk_bc
        scratch = vp.tile([128, 1024], f32, tag="scratch")
        kh = vp.tile([128, 8], f32, tag="kh")       # [c, half] partials
        for h in range(2):
            for c in range(4):
                nc.vector.scalar_tensor_tensor(
                    out=scratch[:, h * 512:(h + 1) * 512],
                    in0=kw_sb[:, c * 1024 + h * 512: c * 1024 + (h + 1) * 512],
                    scalar=1.0, in1=xk_bc[:, h * 512:(h + 1) * 512],
                    op0=Alu.mult, op1=Alu.mult,
                    accum_out=kh[:, 2 * c + h: 2 * c + h + 1])
        k_sb = vp.tile([128, 4], f32, tag="k")
        nc.vector.tensor_add(k_sb[:], kh[:].rearrange("p (c h) -> p c h", h=2)[:, :, 0],
                             kh[:].rearrange("p (c h) -> p c h", h=2)[:, :, 1])
        vr_sb = vp.tile([128, 9], f32, tag="vr")
        ksq = vp.tile([128, 4], f32, tag="ksq")
        # relu lands in `scratch` purely to chain WAW deps: it keeps the
        # in-order DVE from scheduling the rw dot ahead of the k epilogue
        nc.vector.tensor_scalar_max(scratch[:, 0:4], k_sb[:], 0.0)
        nc.vector.tensor_mul(ksq[:], scratch[:, 0:4], scratch[:, 0:4])

        if stage < 5:
            return

        # ---- k broadcast: PE transpose, then 4 selector matmuls into one
        #      PSUM bank (vw dots read it straight from PSUM)
        from concourse.masks import make_identity
        ident = vp.tile([128, 128], f32, tag="ident")
        make_identity(nc, ident)
        kT_ps = pp.tile([4, 128], f32, tag="pmisc", bufs=2)
        nc.tensor.transpose(kT_ps[:], ksq[:], ident[:])
        kT = vp.tile([4, 128], f32, tag="kT")
        nc.scalar.copy(kT[:], kT_ps[:])
        k_bc = pp.tile([128, 512], f32, tag="kbc_ps", bufs=1)
        for c in range(4):
            nc.tensor.matmul(k_bc[:, c * 128:(c + 1) * 128],
                             sel4[:, c * 128:(c + 1) * 128], kT[:],
                             start=True, stop=True)
        if stage < 6:
            return

        # ---- broadcast xr (during kw dots) and compute r
        xr_bc = vp.tile([128, 1024], f32, tag="xr_bc")
        for j in range(8):
            pb = bp.tile([128, 128], f32, tag="pb", name=f"pbr{j}")
            nc.tensor.matmul(pb[:], sel8[:, j * 128:(j + 1) * 128], xr_row[:],
                             start=True, stop=True)
            nc.scalar.copy(xr_bc[:, j * 128:(j + 1) * 128], pb[:])

        pre_r = vp.tile([128, 1], f32, tag="pre_r")
        nc.vector.scalar_tensor_tensor(
            out=scratch[:], in0=rw_sb[:], scalar=1.0, in1=xr_bc[:],
            op0=Alu.mult, op1=Alu.mult, accum_out=pre_r[:])
        nc.scalar.activation(vr_sb[:, 8:9], pre_r[:], Act.Sigmoid)


        # ---- stage V: v partial, 8 d-chunks of [128, 512] x k_bc
        for m in range(8):
            nc.vector.scalar_tensor_tensor(
                out=scratch[:, 0:512], in0=vw_sb[:, m * 512:(m + 1) * 512],
                scalar=1.0, in1=k_bc[:],
                op0=Alu.mult, op1=Alu.mult, accum_out=vr_sb[:, m:m + 1])

        # ---- outputs in row form (contiguous DMA): one transpose via PE
        vrT_ps = pp.tile([9, 128], f32, tag="pmisc", bufs=2)
        nc.tensor.transpose(vrT_ps[:], vr_sb[:], ident[:])
        vrT = vp.tile([9, 128], f32, tag="vrT")
        nc.scalar.copy(vrT[:], vrT_ps[:])
        nc.sync.dma_start(out=vr_d[:], in_=vrT[:])


def _build(stage=6):
    import concourse.bacc as bacc
    import concourse.tile as tile
    from concourse import mybir

    nc = bacc.Bacc("TRN2", target_bir_lowering=False, debug=False,
                   num_devices=N_CORES)
    with tile.TileContext(nc) as tc:
        _body(nc, tc, mybir, stage)
    nc.compile()
    return nc


def _prep_shared(kw, vw, rw):
    """Slice + reshape weights per core (rows onto 128 partitions)."""
    kw_p, vw_p, rw_p = [], [], []
    for i in range(N_CORES):
        A = kw[i * FSH:(i + 1) * FSH, :]                # (512, 1024) rows f
        A = A.reshape(4, 128, 1024).transpose(1, 0, 2)  # [p, c, d]
        kw_p.append(np.ascontiguousarray(A.reshape(128, 4096)))

        B = rw[i * DSH:(i + 1) * DSH, :]                # (128, 1024) rows d
        rw_p.append(np.ascontiguousarray(B))

        C = vw[:, i * FSH:(i + 1) * FSH]                # (1024, 512) rows d
        C = C.reshape(8, 128, FSH).transpose(1, 0, 2)   # [p, m, f]
        vw_p.append(np.ascontiguousarray(C.reshape(128, 4096)))
    return kw_p, vw_p, rw_p


def _prep_smalls(x, state, tmk, tmr, lnw, lnb):
    sm = np.stack([x.reshape(8, 128), state[0].reshape(8, 128),
                   tmk.reshape(8, 128), tmr.reshape(8, 128),
                   lnw.reshape(8, 128), lnb.reshape(8, 128)], axis=1)
    return np.ascontiguousarray(sm.reshape(8, 768))


def kernel(x, state, time_mix_k, time_mix_r, kw, vw, rw, ln_weight, ln_bias):
    from concourse import bass_utils

    x = np.asarray(x, dtype=np.float32)
    state = np.asarray(state, dtype=np.float32)
    kw = np.asarray(kw, dtype=np.float32)
    vw = np.asarray(vw, dtype=np.float32)
    rw = np.asarray(rw, dtype=np.float32)
    tmk = np.asarray(time_mix_k, dtype=np.float32)
    tmr = np.asarray(time_mix_r, dtype=np.float32)
    lnw = np.asarray(ln_weight, dtype=np.float32)
    lnb = np.asarray(ln_bias, dtype=np.float32)

    if "nc" not in _STATE:
        _STATE["nc"] = _build()
    nc = _STATE["nc"]

    kw_p, vw_p, rw_p = _prep_shared(kw, vw, rw)
    sm = _prep_smalls(x, state, tmk, tmr, lnw, lnb)

    in_maps = [{"kw_p": kw_p[i], "vw_p": vw_p[i], "rw_p": rw_p[i], "smalls": sm}
               for i in range(N_CORES)]

    res = bass_utils.run_bass_kernel_spmd(nc, in_maps, core_ids=list(range(N_CORES)))

    # unshard: v = sum of partials, r = concat of chunks
    v = np.zeros(D, dtype=np.float64)
    for i in range(N_CORES):
        v += res.results[i]["vr_out"][:8].reshape(D).astype(np.float64)
    r = np.concatenate([res.results[i]["vr_out"][8]
                        for i in range(N_CORES)])
    out = x + r * v.astype(np.float32)
    xn = res.results[0]["xn_out"].reshape(D)
    return np.asarray(out, dtype=np.float32), np.asarray(xn, dtype=np.float32)


# revision 16
# speedup vs baseline: 1.2518x; 1.0093x over previous
"""RWKV ChannelMixer (single-token) on 8 Trainium2 NeuronCores.

Math (reference):
    xn  = LayerNorm(x) * ln_w + ln_b
    xk  = xn*tmk + prev*(1-tmk);  xr = xn*tmr + prev*(1-tmr)
    r   = sigmoid(rw @ xr)                       # (D,)
    k   = relu(kw @ xk)^2                        # (F,)
    out = x + r * (vw @ k)                       # (D,)
    returns (out, xn)

Sharding (8 cores, no collectives -- cross-core sync costs ~60us here):
    kw: F-row-sharded (512 rows/core)  -> local k chunk (512,)
    vw: F-col-sharded (512 cols/core)  -> partial v_i = vw[:,Fi] @ k_i (1024,)
    rw: D-row-sharded (128 rows/core)  -> r chunk (128,)
    LN/mix replicated.  Host unshard: v = sum_i v_i, r = concat(r_i),
    out = x + r*v.

Engines: dot-products run on the Vector engine (tensor_tensor_reduce,
fp32 @ ~1 elem/lane/cycle ~ 490GB/s > 358GB/s HBM/core, so the kernel
stays DMA-bound).  TensorE only does tiny selector-matmul broadcasts /
transposes.  Weight matrices stream through SBUF in natural row-major
layout (host reshapes rows onto 128 partitions; no transposes).
"""

import sys
import numpy as np

for _p in ("/opt/trn_rl_repo", "/root/.axon_site/_ro/trn_rl_repo"):
    if _p not in sys.path:
        sys.path.append(_p)

D = 1024
F = 4096
N_CORES = 8
FSH = F // N_CORES      # 512 kw rows / vw cols per core
DSH = D // N_CORES      # 128 rw rows per core
LN_EPS = 1e-5

_STATE = {}


def _body(nc, tc, mybir, stage):
    f32 = mybir.dt.float32
    Alu = mybir.AluOpType
    Act = mybir.ActivationFunctionType
    AxX = mybir.AxisListType.X

    kw_d = nc.dram_tensor("kw_p", [128, 4096], f32, kind="ExternalInput").ap()
    vw_d = nc.dram_tensor("vw_p", [128, 4096], f32, kind="ExternalInput").ap()
    rw_d = nc.dram_tensor("rw_p", [128, 1024], f32, kind="ExternalInput").ap()
    # stacked vectors [8, 10*128]: x, prev, ck=tmk*lnw, cr=tmr*lnw,
    # gk=tmk*lnb, gr=tmr*lnb, hk=1-tmk, hr=1-tmr, lnw, lnb
    sm_d = nc.dram_tensor("smalls", [8, 1280], f32, kind="ExternalInput").ap()

    xn_d = nc.dram_tensor("xn_out", [8, 128], f32, kind="ExternalOutput").ap()
    vr_d = nc.dram_tensor("vr_out", [9, 128], f32, kind="ExternalOutput").ap()

    import contextlib
    with contextlib.ExitStack() as ctx:
        wp = ctx.enter_context(tc.tile_pool(name="w", bufs=1))
        vp = ctx.enter_context(tc.tile_pool(name="v", bufs=1))
        bp = ctx.enter_context(tc.tile_pool(name="bc", bufs=2, space="PSUM"))
        pp = ctx.enter_context(tc.tile_pool(name="ps", bufs=1, space="PSUM"))
        dp = ctx.enter_context(tc.tile_pool(name="dr", bufs=1, space="DRAM"))

        # ---- small packed DMA first, then bulk (same HWDGE FIFO: sm->kw->rw->vw)
        sm_sb = vp.tile([8, 1280], f32, tag="sm")
        nc.sync.dma_start(out=sm_sb[:], in_=sm_d[:])
        x_row = sm_sb[:, 0:128]
        pv_row = sm_sb[:, 128:256]
        ck = sm_sb[:, 256:384]
        cr = sm_sb[:, 384:512]
        gk_row = sm_sb[:, 512:640]
        gr_row = sm_sb[:, 640:768]
        hk_row = sm_sb[:, 768:896]
        hr_row = sm_sb[:, 896:1024]
        lw_row = sm_sb[:, 1024:1152]
        lb_row = sm_sb[:, 1152:1280]

        if stage >= 2:
            kw_sb = wp.tile([128, 4096], f32, tag="kw")
            rw_sb = wp.tile([128, 1024], f32, tag="rw")
            vw_sb = wp.tile([128, 4096], f32, tag="vw")
            for c in range(4):
                nc.sync.dma_start(out=kw_sb[:, c * 1024:(c + 1) * 1024],
                                  in_=kw_d[:, c * 1024:(c + 1) * 1024])
            nc.sync.dma_start(out=rw_sb[:], in_=rw_d[:])
            for c in range(4):
                nc.sync.dma_start(out=vw_sb[:, c * 1024:(c + 1) * 1024],
                                  in_=vw_d[:, c * 1024:(c + 1) * 1024])

        # ---- constants
        ones_c8 = vp.tile([8, 1], f32, tag="ones_c8")
        ones_r8 = vp.tile([1, 8], f32, tag="ones_r8")
        eps_t = vp.tile([1, 1], f32, tag="eps")
        eps8 = vp.tile([8, 1], f32, tag="eps8")
        nc.vector.memset(ones_c8[:], 1.0)
        nc.vector.memset(ones_r8[:], 1.0)
        nc.vector.memset(eps_t[:], LN_EPS)
        nc.vector.memset(eps8[:], LN_EPS)
        warm_sqrt = vp.tile([1, 1], f32, tag="warm_sqrt")
        nc.scalar.activation(warm_sqrt[:], eps_t[:], Act.Sqrt)
        if stage >= 3:
            # one-hot row-selector matrices (lhsT for row-broadcast matmuls)
            sel8 = vp.tile([8, 1024], f32, tag="sel8")
            sel4 = vp.tile([4, 512], f32, tag="sel4")
            nc.gpsimd.memset(sel8[:], 0.0)
            nc.gpsimd.memset(sel4[:], 0.0)
            nc.gpsimd.affine_select(
                out=sel8[:].rearrange("p (j q) -> p j q", j=8),
                in_=sel8[:].rearrange("p (j q) -> p j q", j=8),
                compare_op=Alu.not_equal, fill=1.0, base=0,
                pattern=[[-1, 8], [0, 128]], channel_multiplier=1)
            nc.gpsimd.affine_select(
                out=sel4[:].rearrange("p (j q) -> p j q", j=4),
                in_=sel4[:].rearrange("p (j q) -> p j q", j=4),
                compare_op=Alu.not_equal, fill=1.0, base=0,
                pattern=[[-1, 4], [0, 128]], channel_multiplier=1)

        # ---- LayerNorm stats over 1024 elems laid out [8, 128]
        s2 = vp.tile([8, 2], f32, tag="s2")
        xsq = vp.tile([8, 128], f32, tag="xsq")
        nc.vector.tensor_reduce(out=s2[:, 0:1], in_=x_row, axis=AxX, op=Alu.add)
        nc.vector.scalar_tensor_tensor(out=xsq[:], in0=x_row, scalar=1.0,
                                       in1=x_row, op0=Alu.mult, op1=Alu.mult,
                                       accum_out=s2[:, 1:2])

        psum_s = pp.tile([1, 2], f32, tag="pmisc", bufs=2)
        nc.tensor.matmul(psum_s[:], ones_c8[:], s2[:], start=True, stop=True)
        ssum = vp.tile([1, 2], f32, tag="ssum")     # raw [sum_x, sum_x2]
        nc.scalar.copy(ssum[:], psum_s[:])
        psum_b = pp.tile([8, 2], f32, tag="pmisc", bufs=2)
        nc.tensor.matmul(psum_b[:], ones_r8[:], ssum[:], start=True, stop=True)
        bc8 = vp.tile([8, 2], f32, tag="bc8")       # per-part raw sums
        nc.scalar.mul(bc8[:], psum_b[:], 1.0 / D)   # [mean, E[x^2]] per part

        mean8 = bc8[:, 0:1]
        var8 = vp.tile([8, 1], f32, tag="var8")
        std8 = vp.tile([8, 1], f32, tag="std8")
        rstd8 = vp.tile([8, 1], f32, tag="rstd8")
        nc.vector.tensor_mul(var8[:], mean8, mean8)
        nc.vector.tensor_sub(var8[:], bc8[:, 1:2], var8[:])
        nc.scalar.activation(std8[:], var8[:], Act.Sqrt, bias=eps8[:])
        nc.vector.reciprocal(rstd8[:], std8[:])

        # Mix offsets Ek = gk + prev*hk (coefficients ck/gk/hk are pure
        # parameter products, folded on the host).  Emitted here so the
        # in-order DVE runs them inside the PE stats round-trip window.
        ek = vp.tile([8, 128], f32, tag="ek")
        er = vp.tile([8, 128], f32, tag="er")
        nc.vector.tensor_mul(ek[:], pv_row, hk_row)
        nc.vector.tensor_add(ek[:], ek[:], gk_row)
        nc.vector.tensor_mul(er[:], pv_row, hr_row)
        nc.vector.tensor_add(er[:], er[:], gr_row)

        xn_pre = vp.tile([8, 128], f32, tag="xn_pre")
        nc.vector.tensor_scalar(out=xn_pre[:], in0=x_row,
                                scalar1=mean8, scalar2=rstd8[:],
                                op0=Alu.subtract, op1=Alu.mult)

        # ---- token mixes straight from xn_pre (critical path)
        xk_row = vp.tile([8, 128], f32, tag="xk")
        xr_row = vp.tile([8, 128], f32, tag="xr")
        nc.vector.tensor_mul(xk_row[:], xn_pre[:], ck)
        nc.vector.tensor_add(xk_row[:], xk_row[:], ek[:])
        nc.vector.tensor_mul(xr_row[:], xn_pre[:], cr)
        nc.vector.tensor_add(xr_row[:], xr_row[:], er[:])

        # full xn only feeds the output (off the critical path)
        xn_row = vp.tile([8, 128], f32, tag="xn")
        nc.vector.tensor_mul(xn_row[:], xn_pre[:], lw_row)
        nc.vector.tensor_add(xn_row[:], xn_row[:], lb_row)
        nc.sync.dma_start(out=xn_d[:], in_=xn_row[:])

        if stage < 3:
            return

        # ---- broadcast xk across partitions: [8,128] -> [128, 1024]
        xk_bc = vp.tile([128, 1024], f32, tag="xk_bc")
        for j in range(8):
            pb = bp.tile([128, 128], f32, tag="pb", name=f"pbk{j}")
            nc.tensor.matmul(pb[:], sel8[:, j * 128:(j + 1) * 128], xk_row[:],
                             start=True, stop=True)
            nc.scalar.copy(xk_bc[:, j * 128:(j + 1) * 128], pb[:])

        if stage < 4:
            return

        # ---- stage A: k chunk = sqrelu(kw_i @ xk), contraction split in
        #      halves so the first 4 dots overlap the 2nd half of xk_bc
        scratch = vp.tile([128, 1024], f32, tag="scratch")
        kh = vp.tile([128, 8], f32, tag="kh")       # [c, half] partials
        for h in range(2):
            for c in range(4):
                nc.vector.scalar_tensor_tensor(
                    out=scratch[:, h * 512:(h + 1) * 512],
                    in0=kw_sb[:, c * 1024 + h * 512: c * 1024 + (h + 1) * 512],
                    scalar=1.0, in1=xk_bc[:, h * 512:(h + 1) * 512],
                    op0=Alu.mult, op1=Alu.mult,
                    accum_out=kh[:, 2 * c + h: 2 * c + h + 1])
        k_sb = vp.tile([128, 4], f32, tag="k")
        nc.vector.tensor_add(k_sb[:], kh[:].rearrange("p (c h) -> p c h", h=2)[:, :, 0],
                             kh[:].rearrange("p (c h) -> p c h", h=2)[:, :, 1])
        vr_sb = vp.tile([128, 9], f32, tag="vr")
        ksq = vp.tile([128, 4], f32, tag="ksq")
        # relu lands in `scratch` purely to chain WAW deps: it keeps the
        # in-order DVE from scheduling the rw dot ahead of the k epilogue
        nc.vector.tensor_scalar_max(scratch[:, 0:4], k_sb[:], 0.0)
        nc.vector.tensor_mul(ksq[:], scratch[:, 0:4], scratch[:, 0:4])

        if stage < 5:
            return

        # ---- k broadcast: PE transpose, then 4 selector matmuls into one
        #      PSUM bank (vw dots read it straight from PSUM)
        from concourse.masks import make_identity
        ident = vp.tile([128, 128], f32, tag="ident")
        make_identity(nc, ident)
        kT_ps = pp.tile([4, 128], f32, tag="pmisc", bufs=2)
        nc.tensor.transpose(kT_ps[:], ksq[:], ident[:])
        kT = vp.tile([4, 128], f32, tag="kT")
        nc.scalar.copy(kT[:], kT_ps[:])
        k_bc = pp.tile([128, 512], f32, tag="kbc_ps", bufs=1)
        for c in range(4):
            nc.tensor.matmul(k_bc[:, c * 128:(c + 1) * 128],
                             sel4[:, c * 128:(c + 1) * 128], kT[:],
                             start=True, stop=True)
        if stage < 6:
            return

        # ---- broadcast xr (during kw dots) and compute r
        xr_bc = vp.tile([128, 1024], f32, tag="xr_bc")
        for j in range(8):
            pb = bp.tile([128, 128], f32, tag="pb", name=f"pbr{j}")
            nc.tensor.matmul(pb[:], sel8[:, j * 128:(j + 1) * 128], xr_row[:],
                             start=True, stop=True)
            nc.scalar.copy(xr_bc[:, j * 128:(j + 1) * 128], pb[:])

        pre_r = vp.tile([128, 1], f32, tag="pre_r")
        nc.vector.scalar_tensor_tensor(
            out=scratch[:], in0=rw_sb[:], scalar=1.0, in1=xr_bc[:],
            op0=Alu.mult, op1=Alu.mult, accum_out=pre_r[:])
        nc.scalar.activation(vr_sb[:, 8:9], pre_r[:], Act.Sigmoid)


        # ---- stage V: v partial, 8 d-chunks of [128, 512] x k_bc
        for m in range(8):
            nc.vector.scalar_tensor_tensor(
                out=scratch[:, 0:512], in0=vw_sb[:, m * 512:(m + 1) * 512],
                scalar=1.0, in1=k_bc[:],
                op0=Alu.mult, op1=Alu.mult, accum_out=vr_sb[:, m:m + 1])

        # ---- outputs in row form (contiguous DMA): one transpose via PE
        vrT_ps = pp.tile([9, 128], f32, tag="pmisc", bufs=2)
        nc.tensor.transpose(vrT_ps[:], vr_sb[:], ident[:])
        vrT = vp.tile([9, 128], f32, tag="vrT")
        nc.scalar.copy(vrT[:], vrT_ps[:])
        nc.sync.dma_start(out=vr_d[:], in_=vrT[:])


def _build(stage=6):
    import concourse.bacc as bacc
    import concourse.tile as tile
    from concourse import mybir

    nc = bacc.Bacc("TRN2", target_bir_lowering=False, debug=False,
                   num_devices=N_CORES)
    with tile.TileContext(nc) as tc:
        _body(nc, tc, mybir, stage)
    nc.compile()
    return nc


def _prep_shared(kw, vw, rw):
    """Slice + reshape weights per core (rows onto 128 partitions)."""
    kw_p, vw_p, rw_p = [], [], []
    for i in range(N_CORES):
        A = kw[i * FSH:(i + 1) * FSH, :]                # (512, 1024) rows f
        A = A.reshape(4, 128, 1024).transpose(1, 0, 2)  # [p, c, d]
        kw_p.append(np.ascontiguousarray(A.reshape(128, 4096)))

        B = rw[i * DSH:(i + 1) * DSH, :]                # (128, 1024) rows d
        rw_p.append(np.ascontiguousarray(B))

        C = vw[:, i * FSH:(i + 1) * FSH]                # (1024, 512) rows d
        C = C.reshape(8, 128, FSH).transpose(1, 0, 2)   # [p, m, f]
        vw_p.append(np.ascontiguousarray(C.reshape(128, 4096)))
    return kw_p, vw_p, rw_p


def _prep_smalls(x, state, tmk, tmr, lnw, lnb):
    vecs = [x, state[0], tmk * lnw, tmr * lnw, tmk * lnb, tmr * lnb,
            1.0 - tmk, 1.0 - tmr, lnw, lnb]
    sm = np.stack([v.reshape(8, 128) for v in vecs], axis=1)
    return np.ascontiguousarray(sm.reshape(8, 1280))


def kernel(x, state, time_mix_k, time_mix_r, kw, vw, rw, ln_weight, ln_bias):
    from concourse import bass_utils

    x = np.asarray(x, dtype=np.float32)
    state = np.asarray(state, dtype=np.float32)
    kw = np.asarray(kw, dtype=np.float32)
    vw = np.asarray(vw, dtype=np.float32)
    rw = np.asarray(rw, dtype=np.float32)
    tmk = np.asarray(time_mix_k, dtype=np.float32)
    tmr = np.asarray(time_mix_r, dtype=np.float32)
    lnw = np.asarray(ln_weight, dtype=np.float32)
    lnb = np.asarray(ln_bias, dtype=np.float32)

    if "nc" not in _STATE:
        _STATE["nc"] = _build()
    nc = _STATE["nc"]

    kw_p, vw_p, rw_p = _prep_shared(kw, vw, rw)
    sm = _prep_smalls(x, state, tmk, tmr, lnw, lnb)

    in_maps = [{"kw_p": kw_p[i], "vw_p": vw_p[i], "rw_p": rw_p[i], "smalls": sm}
               for i in range(N_CORES)]

    res = bass_utils.run_bass_kernel_spmd(nc, in_maps, core_ids=list(range(N_CORES)))

    # unshard: v = sum of partials, r = concat of chunks
    v = np.zeros(D, dtype=np.float64)
    for i in range(N_CORES):
        v += res.results[i]["vr_out"][:8].reshape(D).astype(np.float64)
    r = np.concatenate([res.results[i]["vr_out"][8]
                        for i in range(N_CORES)])
    out = x + r * v.astype(np.float32)
    xn = res.results[0]["xn_out"].reshape(D)
    return np.asarray(out, dtype=np.float32), np.asarray(xn, dtype=np.float32)


# revision 17
# speedup vs baseline: 1.2526x; 1.0006x over previous
"""RWKV ChannelMixer (single-token) on 8 Trainium2 NeuronCores.

Math (reference):
    xn  = LayerNorm(x) * ln_w + ln_b
    xk  = xn*tmk + prev*(1-tmk);  xr = xn*tmr + prev*(1-tmr)
    r   = sigmoid(rw @ xr)                       # (D,)
    k   = relu(kw @ xk)^2                        # (F,)
    out = x + r * (vw @ k)                       # (D,)
    returns (out, xn)

Sharding (8 cores, no collectives -- cross-core sync costs ~60us here):
    kw: F-row-sharded (512 rows/core)  -> local k chunk (512,)
    vw: F-col-sharded (512 cols/core)  -> partial v_i = vw[:,Fi] @ k_i (1024,)
    rw: D-row-sharded (128 rows/core)  -> r chunk (128,)
    LN/mix replicated.  Host unshard: v = sum_i v_i, r = concat(r_i),
    out = x + r*v.

Engines: dot-products run on the Vector engine (scalar_tensor_tensor
with accum_out = fused multiply + free-dim reduce, fp32 @ ~1 elem/lane/
cycle ~ 490GB/s > 358GB/s HBM/core).  TensorE does tiny selector-matmul
partition-broadcasts and output transposes.  Weights stream through
SBUF in natural row-major layout (host only slices/reshapes; pure
parameter products like tmk*lnw are folded on the host).
"""

import sys
import numpy as np

for _p in ("/opt/trn_rl_repo", "/root/.axon_site/_ro/trn_rl_repo"):
    if _p not in sys.path:
        sys.path.append(_p)

D = 1024
F = 4096
N_CORES = 8
FSH = F // N_CORES      # 512 kw rows / vw cols per core
DSH = D // N_CORES      # 128 rw rows per core
LN_EPS = 1e-5

_STATE = {}


def _body(nc, tc, mybir, stage):
    f32 = mybir.dt.float32
    Alu = mybir.AluOpType
    Act = mybir.ActivationFunctionType
    AxX = mybir.AxisListType.X

    kw_d = nc.dram_tensor("kw_p", [128, 4096], f32, kind="ExternalInput").ap()
    vw_d = nc.dram_tensor("vw_p", [128, 4096], f32, kind="ExternalInput").ap()
    rw_d = nc.dram_tensor("rw_p", [128, 1024], f32, kind="ExternalInput").ap()
    # stacked vectors [8, 10*128]: x, prev, ck=tmk*lnw, cr=tmr*lnw,
    # gk=tmk*lnb, gr=tmr*lnb, hk=1-tmk, hr=1-tmr, lnw, lnb
    sm_d = nc.dram_tensor("smalls", [8, 1280], f32, kind="ExternalInput").ap()

    xn_d = nc.dram_tensor("xn_out", [8, 128], f32, kind="ExternalOutput").ap()
    vr_d = nc.dram_tensor("vr_out", [9, 128], f32, kind="ExternalOutput").ap()

    import contextlib
    with contextlib.ExitStack() as ctx:
        wp = ctx.enter_context(tc.tile_pool(name="w", bufs=1))
        vp = ctx.enter_context(tc.tile_pool(name="v", bufs=1))
        bp = ctx.enter_context(tc.tile_pool(name="bc", bufs=2, space="PSUM"))
        pp = ctx.enter_context(tc.tile_pool(name="ps", bufs=1, space="PSUM"))

        # ---- small packed DMA first, then bulk (same HWDGE FIFO: sm->kw->rw->vw)
        sm_sb = vp.tile([8, 1280], f32, tag="sm")
        nc.sync.dma_start(out=sm_sb[:], in_=sm_d[:])
        x_row = sm_sb[:, 0:128]
        pv_row = sm_sb[:, 128:256]
        ck = sm_sb[:, 256:384]
        cr = sm_sb[:, 384:512]
        gk_row = sm_sb[:, 512:640]
        gr_row = sm_sb[:, 640:768]
        hk_row = sm_sb[:, 768:896]
        hr_row = sm_sb[:, 896:1024]
        lw_row = sm_sb[:, 1024:1152]
        lb_row = sm_sb[:, 1152:1280]

        if stage >= 2:
            kw_sb = wp.tile([128, 4096], f32, tag="kw")
            rw_sb = wp.tile([128, 1024], f32, tag="rw")
            vw_sb = wp.tile([128, 4096], f32, tag="vw")
            for c in range(4):
                nc.sync.dma_start(out=kw_sb[:, c * 1024:(c + 1) * 1024],
                                  in_=kw_d[:, c * 1024:(c + 1) * 1024])
            nc.sync.dma_start(out=rw_sb[:], in_=rw_d[:])
            for c in range(4):
                nc.sync.dma_start(out=vw_sb[:, c * 1024:(c + 1) * 1024],
                                  in_=vw_d[:, c * 1024:(c + 1) * 1024])

        # ---- constants
        ones_c8 = vp.tile([8, 1], f32, tag="ones_c8")
        ones_r8 = vp.tile([1, 8], f32, tag="ones_r8")
        eps_t = vp.tile([1, 1], f32, tag="eps")
        eps8 = vp.tile([8, 1], f32, tag="eps8")
        nc.vector.memset(ones_c8[:], 1.0)
        nc.vector.memset(ones_r8[:], 1.0)
        nc.vector.memset(eps_t[:], LN_EPS)
        nc.vector.memset(eps8[:], LN_EPS)
        warm_sqrt = vp.tile([1, 1], f32, tag="warm_sqrt")
        nc.scalar.activation(warm_sqrt[:], eps_t[:], Act.Sqrt)
        if stage >= 3:
            # one-hot row-selector matrices (lhsT for row-broadcast matmuls)
            sel8 = vp.tile([8, 1024], f32, tag="sel8")
            sel4 = vp.tile([4, 512], f32, tag="sel4")
            nc.gpsimd.memset(sel8[:], 0.0)
            nc.gpsimd.memset(sel4[:], 0.0)
            nc.gpsimd.affine_select(
                out=sel8[:].rearrange("p (j q) -> p j q", j=8),
                in_=sel8[:].rearrange("p (j q) -> p j q", j=8),
                compare_op=Alu.not_equal, fill=1.0, base=0,
                pattern=[[-1, 8], [0, 128]], channel_multiplier=1)
            nc.gpsimd.affine_select(
                out=sel4[:].rearrange("p (j q) -> p j q", j=4),
                in_=sel4[:].rearrange("p (j q) -> p j q", j=4),
                compare_op=Alu.not_equal, fill=1.0, base=0,
                pattern=[[-1, 4], [0, 128]], channel_multiplier=1)

        # ---- LayerNorm stats over 1024 elems laid out [8, 128]
        s2 = vp.tile([8, 2], f32, tag="s2")
        xsq = vp.tile([8, 128], f32, tag="xsq")
        nc.vector.tensor_reduce(out=s2[:, 0:1], in_=x_row, axis=AxX, op=Alu.add)
        nc.vector.scalar_tensor_tensor(out=xsq[:], in0=x_row, scalar=1.0,
                                       in1=x_row, op0=Alu.mult, op1=Alu.mult,
                                       accum_out=s2[:, 1:2])

        psum_s = pp.tile([1, 2], f32, tag="pmisc", bufs=2)
        nc.tensor.matmul(psum_s[:], ones_c8[:], s2[:], start=True, stop=True)
        ssum = vp.tile([1, 2], f32, tag="ssum")     # raw [sum_x, sum_x2]
        nc.scalar.copy(ssum[:], psum_s[:])
        psum_b = pp.tile([8, 2], f32, tag="pmisc", bufs=2)
        nc.tensor.matmul(psum_b[:], ones_r8[:], ssum[:], start=True, stop=True)
        bc8 = vp.tile([8, 2], f32, tag="bc8")       # per-part raw sums
        nc.scalar.mul(bc8[:], psum_b[:], 1.0 / D)   # [mean, E[x^2]] per part

        mean8 = bc8[:, 0:1]
        var8 = vp.tile([8, 1], f32, tag="var8")
        std8 = vp.tile([8, 1], f32, tag="std8")
        rstd8 = vp.tile([8, 1], f32, tag="rstd8")
        nc.vector.tensor_mul(var8[:], mean8, mean8)
        nc.vector.tensor_sub(var8[:], bc8[:, 1:2], var8[:])
        nc.scalar.activation(std8[:], var8[:], Act.Sqrt, bias=eps8[:])
        nc.vector.reciprocal(rstd8[:], std8[:])

        # Mix offsets Ek = gk + prev*hk (coefficients ck/gk/hk are pure
        # parameter products, folded on the host).  Emitted here so the
        # in-order DVE runs them inside the PE stats round-trip window.
        ek = vp.tile([8, 128], f32, tag="ek")
        er = vp.tile([8, 128], f32, tag="er")
        nc.vector.tensor_mul(ek[:], pv_row, hk_row)
        nc.vector.tensor_add(ek[:], ek[:], gk_row)
        nc.vector.tensor_mul(er[:], pv_row, hr_row)
        nc.vector.tensor_add(er[:], er[:], gr_row)

        xn_pre = vp.tile([8, 128], f32, tag="xn_pre")
        nc.vector.tensor_scalar(out=xn_pre[:], in0=x_row,
                                scalar1=mean8, scalar2=rstd8[:],
                                op0=Alu.subtract, op1=Alu.mult)

        # ---- token mixes straight from xn_pre (critical path)
        xk_row = vp.tile([8, 128], f32, tag="xk")
        xr_row = vp.tile([8, 128], f32, tag="xr")
        nc.vector.tensor_mul(xk_row[:], xn_pre[:], ck)
        nc.vector.tensor_add(xk_row[:], xk_row[:], ek[:])
        nc.vector.tensor_mul(xr_row[:], xn_pre[:], cr)
        nc.vector.tensor_add(xr_row[:], xr_row[:], er[:])

        # full xn only feeds the output (off the critical path)
        xn_row = vp.tile([8, 128], f32, tag="xn")
        nc.vector.tensor_mul(xn_row[:], xn_pre[:], lw_row)
        nc.vector.tensor_add(xn_row[:], xn_row[:], lb_row)
        nc.sync.dma_start(out=xn_d[:], in_=xn_row[:])

        if stage < 3:
            return

        # ---- broadcast xk across partitions: [8,128] -> [128, 1024]
        xk_bc = vp.tile([128, 1024], f32, tag="xk_bc")
        for j in range(8):
            pb = bp.tile([128, 128], f32, tag="pb", name=f"pbk{j}")
            nc.tensor.matmul(pb[:], sel8[:, j * 128:(j + 1) * 128], xk_row[:],
                             start=True, stop=True)
            nc.scalar.copy(xk_bc[:, j * 128:(j + 1) * 128], pb[:])

        if stage < 4:
            return

        # ---- stage A: k chunk = sqrelu(kw_i @ xk), contraction split in
        #      halves so the first 4 dots overlap the 2nd half of xk_bc
        scratch = vp.tile([128, 1024], f32, tag="scratch")
        kh = vp.tile([128, 8], f32, tag="kh")       # [c, half] partials
        for h in range(2):
            for c in range(4):
                nc.vector.scalar_tensor_tensor(
                    out=scratch[:, h * 512:(h + 1) * 512],
                    in0=kw_sb[:, c * 1024 + h * 512: c * 1024 + (h + 1) * 512],
                    scalar=1.0, in1=xk_bc[:, h * 512:(h + 1) * 512],
                    op0=Alu.mult, op1=Alu.mult,
                    accum_out=kh[:, 2 * c + h: 2 * c + h + 1])
        k_sb = vp.tile([128, 4], f32, tag="k")
        nc.vector.tensor_add(k_sb[:], kh[:].rearrange("p (c h) -> p c h", h=2)[:, :, 0],
                             kh[:].rearrange("p (c h) -> p c h", h=2)[:, :, 1])
        vr_sb = vp.tile([128, 9], f32, tag="vr")
        ksq = vp.tile([128, 4], f32, tag="ksq")
        # relu lands in `scratch` purely to chain WAW deps: it keeps the
        # in-order DVE from scheduling the rw dot ahead of the k epilogue
        nc.vector.tensor_scalar_max(scratch[:, 0:4], k_sb[:], 0.0)
        nc.vector.tensor_mul(ksq[:], scratch[:, 0:4], scratch[:, 0:4])

        if stage < 5:
            return

        # ---- k broadcast: PE transpose, then 4 selector matmuls into one
        #      PSUM bank (vw dots read it straight from PSUM)
        from concourse.masks import make_identity
        ident = vp.tile([128, 128], f32, tag="ident")
        make_identity(nc, ident)
        kT_ps = pp.tile([4, 128], f32, tag="pmisc", bufs=2)
        nc.tensor.transpose(kT_ps[:], ksq[:], ident[:])
        kT = vp.tile([4, 128], f32, tag="kT")
        nc.scalar.copy(kT[:], kT_ps[:])
        k_bc = pp.tile([128, 512], f32, tag="kbc_ps", bufs=1)
        for c in range(4):
            nc.tensor.matmul(k_bc[:, c * 128:(c + 1) * 128],
                             sel4[:, c * 128:(c + 1) * 128], kT[:],
                             start=True, stop=True)
        if stage < 6:
            return

        # ---- broadcast xr (during kw dots) and compute r
        xr_bc = vp.tile([128, 1024], f32, tag="xr_bc")
        for j in range(8):
            pb = bp.tile([128, 128], f32, tag="pb", name=f"pbr{j}")
            nc.tensor.matmul(pb[:], sel8[:, j * 128:(j + 1) * 128], xr_row[:],
                             start=True, stop=True)
            nc.scalar.copy(xr_bc[:, j * 128:(j + 1) * 128], pb[:])

        pre_r = vp.tile([128, 1], f32, tag="pre_r")
        nc.vector.scalar_tensor_tensor(
            out=scratch[:], in0=rw_sb[:], scalar=1.0, in1=xr_bc[:],
            op0=Alu.mult, op1=Alu.mult, accum_out=pre_r[:])
        nc.scalar.activation(vr_sb[:, 8:9], pre_r[:], Act.Sigmoid)


        # ---- stage V: v partial, 8 d-chunks of [128, 512] x k_bc
        for m in range(8):
            nc.vector.scalar_tensor_tensor(
                out=scratch[:, 0:512], in0=vw_sb[:, m * 512:(m + 1) * 512],
                scalar=1.0, in1=k_bc[:],
                op0=Alu.mult, op1=Alu.mult, accum_out=vr_sb[:, m:m + 1])

        # ---- outputs in row form (contiguous DMA): one transpose via PE
        vrT_ps = pp.tile([9, 128], f32, tag="pmisc", bufs=2)
        nc.tensor.transpose(vrT_ps[:], vr_sb[:], ident[:])
        vrT = vp.tile([9, 128], f32, tag="vrT")
        nc.scalar.copy(vrT[:], vrT_ps[:])
        nc.sync.dma_start(out=vr_d[:], in_=vrT[:])


def _build(stage=6):
    import concourse.bacc as bacc
    import concourse.tile as tile
    from concourse import mybir

    nc = bacc.Bacc("TRN2", target_bir_lowering=False, debug=False,
                   num_devices=N_CORES)
    with tile.TileContext(nc) as tc:
        _body(nc, tc, mybir, stage)
    nc.compile()
    return nc


def _prep_shared(kw, vw, rw):
    """Slice + reshape weights per core (rows onto 128 partitions)."""
    kw_p, vw_p, rw_p = [], [], []
    for i in range(N_CORES):
        A = kw[i * FSH:(i + 1) * FSH, :]                # (512, 1024) rows f
        A = A.reshape(4, 128, 1024).transpose(1, 0, 2)  # [p, c, d]
        kw_p.append(np.ascontiguousarray(A.reshape(128, 4096)))

        B = rw[i * DSH:(i + 1) * DSH, :]                # (128, 1024) rows d
        rw_p.append(np.ascontiguousarray(B))

        C = vw[:, i * FSH:(i + 1) * FSH]                # (1024, 512) rows d
        C = C.reshape(8, 128, FSH).transpose(1, 0, 2)   # [p, m, f]
        vw_p.append(np.ascontiguousarray(C.reshape(128, 4096)))
    return kw_p, vw_p, rw_p


def _prep_smalls(x, state, tmk, tmr, lnw, lnb):
    vecs = [x, state[0], tmk * lnw, tmr * lnw, tmk * lnb, tmr * lnb,
            1.0 - tmk, 1.0 - tmr, lnw, lnb]
    sm = np.stack([v.reshape(8, 128) for v in vecs], axis=1)
    return np.ascontiguousarray(sm.reshape(8, 1280))


def kernel(x, state, time_mix_k, time_mix_r, kw, vw, rw, ln_weight, ln_bias):
    from concourse import bass_utils

    x = np.asarray(x, dtype=np.float32)
    state = np.asarray(state, dtype=np.float32)
    kw = np.asarray(kw, dtype=np.float32)
    vw = np.asarray(vw, dtype=np.float32)
    rw = np.asarray(rw, dtype=np.float32)
    tmk = np.asarray(time_mix_k, dtype=np.float32)
    tmr = np.asarray(time_mix_r, dtype=np.float32)
    lnw = np.asarray(ln_weight, dtype=np.float32)
    lnb = np.asarray(ln_bias, dtype=np.float32)

    if "nc" not in _STATE:
        _STATE["nc"] = _build()
    nc = _STATE["nc"]

    kw_p, vw_p, rw_p = _prep_shared(kw, vw, rw)
    sm = _prep_smalls(x, state, tmk, tmr, lnw, lnb)

    in_maps = [{"kw_p": kw_p[i], "vw_p": vw_p[i], "rw_p": rw_p[i], "smalls": sm}
               for i in range(N_CORES)]

    res = bass_utils.run_bass_kernel_spmd(nc, in_maps, core_ids=list(range(N_CORES)))

    # unshard: v = sum of partials, r = concat of chunks
    v = np.zeros(D, dtype=np.float64)
    for i in range(N_CORES):
        v += res.results[i]["vr_out"][:8].reshape(D).astype(np.float64)
    r = np.concatenate([res.results[i]["vr_out"][8]
                        for i in range(N_CORES)])
    out = x + r * v.astype(np.float32)
    xn = res.results[0]["xn_out"].reshape(D)
    return np.asarray(out, dtype=np.float32), np.asarray(xn, dtype=np.float32)
